# revision 15
# baseline (speedup 1.0000x reference)
"""TRN2 Bass kernel for nn_Attention_76802605187492.

Math (B=64, T=512, H=1024, A=300):
  The aspect branch only adds a per-batch constant to the attention
  scores, which softmax cancels, so it does not affect the output.
  Per batch b:
    scores[t] = u . tanh(W_h hidden[b,t] + b_h)      u = w_w[0, :H]
    alpha     = softmax_t(scores)
    r         = sum_t alpha[t] hidden[b,t]
    p_b       = r @ W_p.T
    x_j       = hidden[j,-1] @ W_x.T                  (all j)
    out[b,j]  = tanh(p_b + x_j + (b_p + b_x))         -> [B, B, H]

Sharding: data-parallel over batch across 8 cores (8 batches each).

v4 design (vs 139us baseline):
  - big matmul, scores, r and p all in fp8 DoubleRow (2 k-rows/cycle).
    Scores pair m-tiles on the j dim (tanh writes fp8 tz directly);
    r contracts t via an fp8 t-layout upload ht8 (t = (tt2*2+j)*128+p),
    alpha transposed into masked am8 columns, 4 chunks per batch.
    DR ldweights requires the j-plane stride to be >= 16 elements
    (u8/rT8/am8 padded accordingly).
  - batches processed in waves of 2. The SP engine issues DMA
    descriptors at ~1us each and the Tile scheduler hoists dep-free
    issues, so ALL inputs ride the single sync queue as ~19 large DMAs
    in consumption-priority order (the queue executes descriptors
    FIFO); outputs go on the scalar queue.
  - boundary work (r-burst/rT/p/G) is interleaved one step per
    m-iteration into the NEXT wave's matmul stream so the PE never
    idles long enough for the HAM clock-gate to throttle it.
  - x2 (hlast @ W_x.T, bf16 hi@hi + lo@hi; bias rides via k=1 ones
    matmuls) is issued right after wave 0 where the PE is DMA-starved.
  - output pipelined in 3 groups (batches 0-3, 4-5, 6-7): each group's
    r-burst/rT/p/G/out-DMA runs right after its last wave, so only the
    last 2 batches' output (0.26MB) drains at the end.
  - softmax exp uses accum_out to fuse the row-sum.
  - psum: z x4 (2KB slots, also used by boundary transposes), s x2,
    aux x2 = exactly 8 banks.
"""

import os
import sys

sys.path.insert(0, "/opt/trn_rl_repo")
sys.path.insert(0, "/opt/trn_rl_repo/concourse")

import numpy as np
import ml_dtypes

import concourse.bass as bass
import concourse.mybir as mybir
from concourse import tile
from concourse.bass_utils import run_bass_kernel_spmd

F32 = mybir.dt.float32
BF16 = mybir.dt.bfloat16
BF16_NP = ml_dtypes.bfloat16
TANH = mybir.ActivationFunctionType.Tanh
EXP = mybir.ActivationFunctionType.Exp
FP8 = mybir.dt.float8e4
FP8_NP = ml_dtypes.float8_e4m3
DR = mybir.MatmulPerfMode.DoubleRow
WSCALE = 16.0     # W_h pre-scale into fp8 range
USCALE = 128.0    # u pre-scale into fp8 range
ASCALE = 128.0    # alpha pre-scale (max alpha=1 -> 128 < 240 fp8 max)
RS = 32.0         # r pre-scale into fp8 range
WPS = 16.0        # W_p pre-scale into fp8 range

B, T, H = 64, 512, 1024
NCORES = 8
PB = B // NCORES          # batches per core = 8
KT = H // 128             # 8 k-tiles over h_in
MT = H // 128             # 8 m-tiles over h_out
KT2 = H // 256            # 4 double-row k-tiles
TT2 = T // 256            # 2 double-row t-tiles for r
GROUPS = [(0, 4), (4, 2), (6, 2)]   # (first batch, size) output groups

_CACHE: dict = {}


def _build_nc() -> bass.Bass:
    nc = bass.Bass()

    xQ8 = nc.declare_dram_parameter("xQ8", [PB, 128, KT2 * 2 * T], FP8, isOutput=False)
    whQ8 = nc.declare_dram_parameter(
        "whQ8", [MT, 128, KT2 * 2 * 128], FP8, isOutput=False
    )
    ht8 = nc.declare_dram_parameter("ht8", [PB, 128, TT2 * 2 * H], FP8, isOutput=False)
    blobA = nc.declare_dram_parameter("blobA", [128, 64], mybir.dt.uint8,
                                      isOutput=False)
    blobB = nc.declare_dram_parameter("blobB", [4, 520], mybir.dt.uint8,
                                      isOutput=False)
    blobC = nc.declare_dram_parameter("blobC", [1, 4352], mybir.dt.uint8,
                                      isOutput=False)
    wpT8 = nc.declare_dram_parameter("wpT8", [128, KT2 * 2 * H], FP8, isOutput=False)
    wxh = nc.declare_dram_parameter("wxT_hi", [H, H], BF16, isOutput=False)
    hl2 = nc.declare_dram_parameter("hl2", [H, 2 * B], BF16, isOutput=False)
    out = nc.declare_dram_parameter("out", [PB, B, H], BF16, isOutput=True)

    with tile.TileContext(nc) as tc:
        with (
            tc.tile_pool(name="const", bufs=1) as cp,
            tc.tile_pool(name="xchunk", bufs=1) as xp,
            tc.tile_pool(name="hts", bufs=1) as hp,
            tc.tile_pool(name="tz", bufs=6) as tzp,
            tc.tile_pool(name="small", bufs=1) as sp,
            tc.tile_pool(name="sc", bufs=2) as scp,
            tc.tile_pool(name="outp", bufs=4) as op_,
            tc.tile_pool(name="zps", bufs=4, space=bass.MemorySpace.PSUM) as zp,
            tc.tile_pool(name="sps", bufs=2, space=bass.MemorySpace.PSUM) as spp,
            tc.tile_pool(name="aux", bufs=2, space=bass.MemorySpace.PSUM) as ap,
        ):
            # ---- input DMAs: few, large; sync queue in consumption order ----
            blobA_sb = cp.tile([128, 64], mybir.dt.uint8)
            nc.sync.dma_start(blobA_sb[:], blobA[:])
            bh_sb = blobA_sb[:, 0:32].bitcast(F32)
            u8_sb = blobA_sb[:, 32:64].bitcast(FP8).rearrange(
                "p (j q) -> p j q", j=2
            )
            blobB_sb = cp.tile([4, 520], mybir.dt.uint8)
            nc.sync.dma_start(blobB_sb[:], blobB[:])
            selg_sb = blobB_sb[:, 0:512].bitcast(BF16).rearrange(
                "g (q m) -> g q m", q=2
            )
            id_sb = blobB_sb[:, 512:520].bitcast(BF16)
            blobC_sb = cp.tile([1, 4352], mybir.dt.uint8)
            nc.sync.dma_start(blobC_sb[:], blobC[:])
            bpx_sb = blobC_sb[:, 0:4096].bitcast(BF16)
            ones_sb = blobC_sb[:, 4096:4224].bitcast(BF16)

            wm_sb = []

            def _load_wm(m):
                wm = cp.tile([128, KT2, 2, 128], FP8, name=f"wm{m}")
                nc.sync.dma_start(
                    wm[:], whQ8[m].rearrange("p (kt j o) -> p kt j o", j=2, o=128)
                )
                return wm

            xc_sb = []

            def _load_xc(b, split=False):
                xc = xp.tile([128, KT2, 2, T], FP8, name=f"xc{b}")
                halves = ((0, 2), (2, 4)) if split else ((0, 4),)
                for lo, hi in halves:
                    nc.sync.dma_start(
                        xc[:, lo:hi],
                        xQ8[b].rearrange("p (kt j n) -> p kt j n", j=2, n=T)[
                            :, lo:hi
                        ],
                    )
                return xc

            ht_sb = [None] * PB

            wm_sb.append(_load_wm(0))
            xc_sb.append(_load_xc(0, split=True))
            xc_sb.append(_load_xc(1))
            wmrest = cp.tile([128, MT - 1, KT2, 2, 128], FP8)
            nc.sync.dma_start(
                wmrest[:],
                whQ8[1:].rearrange("m p (kt j o) -> p m kt j o", j=2, o=128),
            )
            for m in range(1, MT):
                wm_sb.append(wmrest[:, m - 1])
            for b in (2, 3):
                xc_sb.append(_load_xc(b))
            wxh_sb = cp.tile([128, KT, H], BF16)
            nc.sync.dma_start(wxh_sb[:], wxh[:].rearrange("(kt p) n -> p kt n", p=128))
            hl_sb = cp.tile([128, KT, 2 * B], BF16)
            nc.sync.dma_start(hl_sb[:], hl2[:].rearrange("(kt p) j -> p kt j", p=128))
            hlh_sb = hl_sb[:, :, :B]
            hll_sb = hl_sb[:, :, B:]
            for b in (4, 5):
                xc_sb.append(_load_xc(b))
            htpair = []

            def _load_htpair(hp2):
                htp = hp.tile([128, 2, TT2, 2, H], FP8, name=f"htp{hp2}")
                nc.sync.dma_start(
                    htp[:],
                    ht8[2 * hp2 : 2 * hp2 + 2].rearrange(
                        "b p (tt j h) -> p b tt j h", j=2, h=H
                    ),
                )
                htpair.append(htp)
                ht_sb[2 * hp2] = htp[:, 0]
                ht_sb[2 * hp2 + 1] = htp[:, 1]

            _load_htpair(0)
            _load_htpair(1)
            for b in (6, 7):
                xc_sb.append(_load_xc(b))
            _load_htpair(2)
            _load_htpair(3)
            wpT_sb = cp.tile([128, KT2, 2, H], FP8)
            nc.sync.dma_start(
                wpT_sb[:], wpT8[:].rearrange("p (kt j n) -> p kt j n", j=2, n=H)
            )

            am8 = [None] * len(GROUPS)
            x2_sb = sp.tile([128, H], F32)
            s_ps = [None, None]

            def softmax_alpha(b, g, bl):
                """s_ps[b%2] -> alpha (scalar/DVE now); returns the deferred
                PE-transpose step so the PE-side work lands in the next
                wave's matmul stream instead of stalling at wave end."""
                e_b = scp.tile([1, T], F32, tag="eb")
                esum = scp.tile([1, 1], F32, tag="es")
                nc.scalar.activation(
                    e_b[:1], s_ps[b % 2][:1], EXP, scale=1.0 / USCALE,
                    accum_out=esum[:1],
                )
                einv = scp.tile([1, 1], F32, tag="ei")
                nc.vector.reciprocal(einv[:1], esum[:1])
                a8 = scp.tile([1, T], BF16, tag="ab")
                nc.vector.tensor_scalar(
                    a8[:1],
                    e_b[:1],
                    einv[:1, :1],
                    ASCALE,
                    mybir.AluOpType.mult,
                    mybir.AluOpType.mult,
                )

                def aT_step():
                    # t = (tt2*2 + jj)*128 + p: chunk c -> col bl of block bl
                    for c in range(4):
                        tp_ps = zp.tile([128, 1], BF16, tag="z", name="tp")
                        nc.tensor.transpose(
                            tp_ps[:, :1],
                            a8[:1, c * 128 : (c + 1) * 128],
                            id_sb[:1, :1],
                        )
                        nc.vector.tensor_scalar_mul(
                            am8[g][:, bl, c // 2, c % 2, bl : bl + 1],
                            tp_ps[:, :1],
                            1.0,
                        )

                return aT_step

            def emit_x2(hc):
                """x2 = hlast @ W_x.T + (b_p + b_x), bf16 hi@hi + lo@hi."""
                if True:
                    x_ps = ap.tile([B, 512], F32, tag="aux", name=f"x{hc}")
                    n = 0
                    nmm = 2 * KT + 2
                    for lh in (hlh_sb, hll_sb):
                        for kt in range(KT):
                            nc.tensor.matmul(
                                x_ps[:],
                                lh[:, kt, :],
                                wxh_sb[:, kt, hc * 512 : (hc + 1) * 512],
                                start=(n == 0),
                                stop=(n == nmm - 1),
                            )
                            n += 1
                    for row in range(2):
                        nc.tensor.matmul(
                            x_ps[:],
                            ones_sb[:1, :],
                            bpx_sb[:1, row * H + hc * 512 : row * H + (hc + 1) * 512],
                            start=(n == 0),
                            stop=(n == nmm - 1),
                        )
                        n += 1
                    nc.vector.tensor_scalar_mul(
                        x2_sb[:B, hc * 512 : (hc + 1) * 512], x_ps[:], 1.0
                    )
                    nc.vector.tensor_scalar_mul(
                        x2_sb[B:, hc * 512 : (hc + 1) * 512], x_ps[:], 1.0
                    )

            def boundary_steps(g):
                """r-burst/rT/p/G for group g as steps interleavable into
                the next wave's m-loop."""
                start, size = GROUPS[g]
                st = {}

                def emit_r(hc):
                    r_ps = ap.tile([4, 512], F32, tag="aux", name=f"r{g}{hc}")
                    n = 0
                    for bl in range(size):
                        for tt in range(TT2):
                            nc.tensor.matmul(
                                r_ps[:size],
                                am8[g][:, bl, tt, :, :size],
                                ht_sb[start + bl][:, tt, :, hc * 512 : (hc + 1) * 512],
                                start=(n == 0),
                                stop=(n == 2 * size - 1),
                                perf_mode=DR,
                            )
                            n += 1
                    rflat = sp.tile([4, 512], BF16, tag="rf", name="rflat", bufs=4)
                    nc.vector.tensor_scalar_mul(rflat[:size], r_ps[:size], RS / ASCALE)
                    st[f"rfl{hc}"] = rflat

                def emit_rT():
                    # rT: h = (kt2*2+j)*128 + p = (hc*4 + ktl)*128 + p
                    rT8 = sp.tile(
                        [128, KT2, 2, 16], FP8, tag="rT", name=f"rT{g}", bufs=2
                    )
                    tps = []
                    for hc in range(2):
                        for ktl in range(4):
                            tp_ps = zp.tile([128, 4], BF16, tag="z", name="tpr")
                            nc.tensor.transpose(
                                tp_ps[:, :size],
                                st[f"rfl{hc}"][:size, ktl * 128 : (ktl + 1) * 128],
                                id_sb[:size, :size],
                            )
                            tps.append((hc * 4 + ktl, tp_ps))
                    for kk, tp_ps in tps:
                        nc.vector.tensor_scalar_mul(
                            rT8[:, kk // 2, kk % 2, :size], tp_ps[:, :size], 1.0
                        )
                    st["rT8"] = rT8

                def emit_p():
                    p_sb = sp.tile([4, H], BF16, tag="psb", name=f"p{g}", bufs=2)
                    for hc in range(2):
                        p_ps = ap.tile([4, 512], F32, tag="aux", name=f"pp{g}{hc}")
                        for kt in range(KT2):
                            nc.tensor.matmul(
                                p_ps[:size],
                                st["rT8"][:, kt, :, :size],
                                wpT_sb[:, kt, :, hc * 512 : (hc + 1) * 512],
                                start=(kt == 0),
                                stop=(kt == KT2 - 1),
                                perf_mode=DR,
                            )
                        nc.vector.tensor_scalar_mul(
                            p_sb[:size, hc * 512 : (hc + 1) * 512],
                            p_ps[:size],
                            1.0 / (RS * WPS),
                        )
                    st["p_sb"] = p_sb

                def emit_G(q):
                    for hc in range(2):
                        o_ps = ap.tile([128, 512], F32, tag="aux", name=f"o{g}{q}{hc}")
                        nc.tensor.matmul(
                            o_ps[:],
                            selg_sb[:size, q, :],
                            st["p_sb"][:size, hc * 512 : (hc + 1) * 512],
                            start=True,
                            stop=True,
                        )
                        o_sb = op_.tile([128, 512], F32, tag="oadd")
                        nc.vector.tensor_tensor(
                            o_sb[:],
                            o_ps[:],
                            x2_sb[:, hc * 512 : (hc + 1) * 512],
                            mybir.AluOpType.add,
                        )
                        o2 = op_.tile([128, 512], BF16, tag="otanh")
                        nc.scalar.activation(o2[:], o_sb[:], TANH)
                        i0 = start + 2 * q
                        nc.scalar.dma_start(
                            out[i0 : i0 + 2, :, hc * 512 : (hc + 1) * 512].rearrange(
                                "i j h -> (i j) h"
                            ),
                            o2[:],
                        )

                steps = [lambda: emit_r(0), lambda: emit_r(1), emit_rT, emit_p]
                for q in range(size // 2):
                    steps.append(lambda q=q: emit_G(q))
                return steps

            # ---- main loop: waves of 2 batches; boundary/x2 steps are
            #      interleaved one per m-iteration to keep the PE dense ----
            pending = []
            for w in range(PB // 2):
                b0 = 2 * w
                g = next(i for i, (s, n) in enumerate(GROUPS) if s <= b0 < s + n)
                gstart, gsize = GROUPS[g]
                if b0 == gstart:
                    am = sp.tile(
                        [128, 4, TT2, 2, 16], FP8, tag=f"am{g}", name=f"am8_{g}"
                    )  # [p, bl, tt2, j, col16]: j-stride 16 for DR ldweights
                    nc.vector.memset(am[:], 0.0)
                    am8[g] = am
                s_ps[0] = spp.tile([1, T], F32, tag="s", name=f"s{b0}")
                s_ps[1] = spp.tile([1, T], F32, tag="s", name=f"s{b0 + 1}")
                tz8 = [None, None]
                for m in range(MT):
                    if pending:
                        pending.pop(0)()
                    z_ps = [
                        zp.tile([128, T], F32, tag="z", name=f"z{b0 + b2}_{m}")
                        for b2 in range(2)
                    ]
                    for kt in range(KT2):
                        for b2 in range(2):
                            nc.tensor.matmul(
                                z_ps[b2][:],
                                wm_sb[m][:, kt, :, :],
                                xc_sb[b0 + b2][:, kt, :, :],
                                start=(kt == 0),
                                stop=(kt == KT2 - 1),
                                perf_mode=DR,
                            )
                    q, jj = m // 2, m % 2
                    for b2 in range(2):
                        if jj == 0:
                            tz8[b2] = tzp.tile(
                                [128, 2, T], FP8, tag="tz8", name=f"tz{b0+b2}_{q}"
                            )
                        nc.scalar.activation(
                            tz8[b2][:, jj, :],
                            z_ps[b2][:],
                            TANH,
                            bias=bh_sb[:, m : m + 1],
                            scale=1.0 / WSCALE,
                        )
                    if jj == 1:
                        for b2 in range(2):
                            nc.tensor.matmul(
                                s_ps[b2][:1, :],
                                u8_sb[:, :, q : q + 1],
                                tz8[b2][:],
                                start=(q == 0),
                                stop=(q == KT2 - 1),
                                perf_mode=DR,
                            )
                for step in pending:
                    step()
                pending = []
                for b2 in range(2):
                    b = b0 + b2
                    pending.append(softmax_alpha(b, g, b - gstart))
                if w == 0:
                    pending += [lambda: emit_x2(0), lambda: emit_x2(1)]
                if b0 + 2 == gstart + gsize:
                    pending.extend(boundary_steps(g))
            for step in pending:
                step()
    _split_excess_waits(nc)
    return nc


def _split_excess_waits(nc: bass.Bass, max_waits: int = 1) -> None:
    """Walrus's per-instruction sync-wait slots are limited; move excess
    on_wait entries onto wait-only NoOps inserted just before the
    instruction (same engine, so ordering is preserved)."""
    for fn in nc.m.functions:
        for blk in fn.blocks:
            new = []
            for inst in blk.instructions:
                si = inst.sync_info
                waits = list(si.on_wait) if si is not None and si.on_wait else []
                if len(waits) > max_waits:
                    extra, keep = waits[:-max_waits], waits[-max_waits:]
                    for ci in range(0, len(extra), max_waits):
                        nop = mybir.InstNoOp(
                            name=f"{inst.name}-wsplit{ci}", ins=[], outs=[]
                        )
                        nop.engine = inst.engine
                        nop.sync_info = mybir.SyncInfo(
                            on_wait=extra[ci : ci + max_waits], on_update=[]
                        )
                        new.append(nop)
                    inst.sync_info = mybir.SyncInfo(
                        on_wait=keep, on_update=list(si.on_update or [])
                    )
                new.append(inst)
            blk.instructions[:] = new


def _split_bf16(a: np.ndarray) -> tuple[np.ndarray, np.ndarray]:
    hi = a.astype(BF16_NP)
    lo = (a - hi.astype(np.float32)).astype(BF16_NP)
    return hi, lo


def _pad_u8(w_w: np.ndarray) -> np.ndarray:
    u = np.zeros((128, 2, 16), np.float32)
    u[:, :, :KT2] = (w_w[0, :H] * USCALE).reshape(KT2, 2, 128).transpose(2, 1, 0)
    return np.ascontiguousarray(u.reshape(128, 2 * 16)).astype(FP8_NP)


def _host_prep(inputs: dict) -> list[dict]:
    hidden = np.asarray(inputs["hidden"], np.float32)
    W_h = np.asarray(inputs["W_h"], np.float32)
    b_h = np.asarray(inputs["b_h"], np.float32)
    w_w = np.asarray(inputs["w_w"], np.float32)
    W_p = np.asarray(inputs["W_p"], np.float32)
    b_p = np.asarray(inputs["b_p"], np.float32)
    W_x = np.asarray(inputs["W_x"], np.float32)
    b_x = np.asarray(inputs["b_x"], np.float32)

    # G selector: row i = 2q + m//64 (q=0 block alone serves 2-row groups)
    selgm = np.zeros((4, 2, 128), np.float32)
    for q in range(2):
        for m in range(128):
            selgm[2 * q + m // 64, q, m] = 1.0

    wxT = np.ascontiguousarray(W_x.T)
    wx_hi, _ = _split_bf16(wxT)
    hlT = np.ascontiguousarray(hidden[:, -1, :].T)
    hl_hi, hl_lo = _split_bf16(hlT)
    bpx_hi, bpx_lo = _split_bf16((b_p + b_x).reshape(1, H))

    bhm = np.ascontiguousarray(b_h.reshape(MT, 128).T, np.float32)
    blobA = np.concatenate(
        [bhm.view(np.uint8).reshape(128, 32),
         _pad_u8(w_w).view(np.uint8).reshape(128, 32)], axis=1
    )
    selg_b = selgm.reshape(4, 2 * 128).astype(BF16_NP)
    ident_b = np.eye(4, dtype=np.float32).astype(BF16_NP)
    blobB = np.concatenate(
        [selg_b.view(np.uint8).reshape(4, 512),
         ident_b.view(np.uint8).reshape(4, 8)], axis=1
    )
    bpx_b = np.concatenate([bpx_hi, bpx_lo], axis=1)
    ones_b = np.ones((1, B), BF16_NP)
    blobC = np.concatenate(
        [bpx_b.view(np.uint8).reshape(1, 4096),
         ones_b.view(np.uint8).reshape(1, 128),
         np.zeros((1, 128), np.uint8)], axis=1
    )
    shared = {
        "whQ8": np.ascontiguousarray(
            (W_h.T * WSCALE)
            .reshape(KT2, 128, 2, MT, 128)
            .transpose(3, 1, 0, 2, 4)
            .reshape(MT, 128, KT2 * 2 * 128)
        ).astype(FP8_NP),
        "blobA": blobA,
        "blobB": blobB,
        "blobC": blobC,
        # wpT8[p, kt2, j, n] = W_p.T[(kt2*2+j)*128 + p, n] * WPS
        "wpT8": np.ascontiguousarray(
            (W_p.T * WPS).reshape(KT2, 2, 128, H).transpose(2, 0, 1, 3).reshape(
                128, KT2 * 2 * H
            )
        ).astype(FP8_NP),
        "wxT_hi": wx_hi,
        "hl2": np.ascontiguousarray(np.concatenate([hl_hi, hl_lo], axis=1)),
    }

    in_maps = []
    for c in range(NCORES):
        slab = hidden[c * PB : (c + 1) * PB]          # [PB, T, H]
        m = dict(shared)
        m["xQ8"] = np.ascontiguousarray(
            slab.reshape(PB, T, KT2, 128, 2)
            .transpose(0, 3, 2, 4, 1)
            .reshape(PB, 128, KT2 * 2 * T)
        ).astype(FP8_NP)
        # ht8[b][p, tt2, j, h] = hidden[b, t = (tt2*2+j)*128 + p, h]
        m["ht8"] = np.ascontiguousarray(
            slab.reshape(PB, TT2, 2, 128, H)
            .transpose(0, 3, 1, 2, 4)
            .reshape(PB, 128, TT2 * 2 * H)
        ).astype(FP8_NP)
        in_maps.append(m)
    return in_maps


def _ensure_ntff_hook() -> None:
    """The agent image's antenv lacks axon_hooks; register a shim module
    wired to the libaxon NTFF profile hook so trace=True works."""
    try:
        from antenv.axon_hooks import get_axon_ntff_profile_hook  # noqa: F401
        return
    except ImportError:
        pass
    import types
    import antenv
    from trn_agent_boot.trn_boot import _ntff_profile_via_ctypes

    mod = types.ModuleType("antenv.axon_hooks")
    holder = {"hook": _ntff_profile_via_ctypes("/opt/axon/libaxon_pjrt.so")}
    mod.get_axon_ntff_profile_hook = lambda: holder["hook"]
    mod.set_axon_ntff_profile_hook = lambda h: holder.__setitem__("hook", h)
    sys.modules["antenv.axon_hooks"] = mod
    antenv.axon_hooks = mod


def run(inputs: dict, trace: bool = False, **kw):
    if trace:
        _ensure_ntff_hook()
    if "nc" not in _CACHE:
        _CACHE["nc"] = _build_nc()
    nc = _CACHE["nc"]
    in_maps = _host_prep(inputs)
    res = run_bass_kernel_spmd(nc, in_maps, list(range(NCORES)), trace=trace, **kw)
    out = np.empty((B, B, H), np.float32)
    for c in range(NCORES):
        out[c * PB : (c + 1) * PB] = np.asarray(res.results[c]["out"], np.float32)
    return out, res


def kernel(**inputs) -> np.ndarray:
    out, _ = run(inputs)
    return out


# revision 17
# speedup vs baseline: 1.0063x; 1.0063x over previous
"""TRN2 Bass kernel for nn_Attention_76802605187492.

Math (B=64, T=512, H=1024, A=300):
  The aspect branch only adds a per-batch constant to the attention
  scores, which softmax cancels, so it does not affect the output.
  Per batch b:
    scores[t] = u . tanh(W_h hidden[b,t] + b_h)      u = w_w[0, :H]
    alpha     = softmax_t(scores)
    r         = sum_t alpha[t] hidden[b,t]
    p_b       = r @ W_p.T
    x_j       = hidden[j,-1] @ W_x.T                  (all j)
    out[b,j]  = tanh(p_b + x_j + (b_p + b_x))         -> [B, B, H]

Sharding: data-parallel over batch across 8 cores (8 batches each).

v4 design (vs 139us baseline):
  - big matmul, scores, r and p all in fp8 DoubleRow (2 k-rows/cycle).
    Scores pair m-tiles on the j dim (tanh writes fp8 tz directly);
    r contracts t via an fp8 t-layout upload ht8 (t = (tt2*2+j)*128+p),
    alpha transposed into masked am8 columns, 4 chunks per batch.
    DR ldweights requires the j-plane stride to be >= 16 elements
    (u8/rT8/am8 padded accordingly).
  - batches processed in waves of 2. The SP engine issues DMA
    descriptors at ~1us each and the Tile scheduler hoists dep-free
    issues, so ALL inputs ride the single sync queue as ~19 large DMAs
    in consumption-priority order (the queue executes descriptors
    FIFO); outputs go on the scalar queue.
  - boundary work (r-burst/rT/p/G) is interleaved one step per
    m-iteration into the NEXT wave's matmul stream so the PE never
    idles long enough for the HAM clock-gate to throttle it.
  - x2 (hlast @ W_x.T, bf16 hi@hi + lo@hi; bias rides via k=1 ones
    matmuls) is issued right after wave 0 where the PE is DMA-starved.
  - output pipelined in 3 groups (batches 0-3, 4-5, 6-7): each group's
    r-burst/rT/p/G/out-DMA runs right after its last wave, so only the
    last 2 batches' output (0.26MB) drains at the end.
  - softmax exp uses accum_out to fuse the row-sum.
  - psum: z x4 (2KB slots, also used by boundary transposes), s x2,
    aux x2 = exactly 8 banks.
"""

import os
import sys

sys.path.insert(0, "/opt/trn_rl_repo")
sys.path.insert(0, "/opt/trn_rl_repo/concourse")

import numpy as np
import ml_dtypes

import concourse.bass as bass
import concourse.mybir as mybir
from concourse import tile
from concourse.bass_utils import run_bass_kernel_spmd

F32 = mybir.dt.float32
BF16 = mybir.dt.bfloat16
BF16_NP = ml_dtypes.bfloat16
TANH = mybir.ActivationFunctionType.Tanh
EXP = mybir.ActivationFunctionType.Exp
FP8 = mybir.dt.float8e4
FP8_NP = ml_dtypes.float8_e4m3
DR = mybir.MatmulPerfMode.DoubleRow
WSCALE = 16.0     # W_h pre-scale into fp8 range
USCALE = 128.0    # u pre-scale into fp8 range
ASCALE = 128.0    # alpha pre-scale (max alpha=1 -> 128 < 240 fp8 max)
RS = 32.0         # r pre-scale into fp8 range
ESC = 8.0         # exp pre-scale into fp8 range (e in (0, ~13))
WPS = 16.0        # W_p pre-scale into fp8 range

B, T, H = 64, 512, 1024
NCORES = 8
PB = B // NCORES          # batches per core = 8
KT = H // 128             # 8 k-tiles over h_in
MT = H // 128             # 8 m-tiles over h_out
KT2 = H // 256            # 4 double-row k-tiles
TT2 = T // 256            # 2 double-row t-tiles for r
GROUPS = [(0, 4), (4, 2), (6, 2)]   # (first batch, size) output groups

_CACHE: dict = {}


def _build_nc() -> bass.Bass:
    nc = bass.Bass()

    xQ8 = nc.declare_dram_parameter("xQ8", [PB, 128, KT2 * 2 * T], FP8, isOutput=False)
    whQ8 = nc.declare_dram_parameter(
        "whQ8", [MT, 128, KT2 * 2 * 128], FP8, isOutput=False
    )
    ht8 = nc.declare_dram_parameter("ht8", [PB, 128, TT2 * 2 * H], FP8, isOutput=False)
    blobA = nc.declare_dram_parameter("blobA", [128, 64], mybir.dt.uint8,
                                      isOutput=False)
    blobB = nc.declare_dram_parameter("blobB", [4, 520], mybir.dt.uint8,
                                      isOutput=False)
    blobC = nc.declare_dram_parameter("blobC", [1, 4352], mybir.dt.uint8,
                                      isOutput=False)
    wpT8 = nc.declare_dram_parameter("wpT8", [128, KT2 * 2 * H], FP8, isOutput=False)
    wxh = nc.declare_dram_parameter("wxT_hi", [H, H], BF16, isOutput=False)
    hl2 = nc.declare_dram_parameter("hl2", [H, 2 * B], BF16, isOutput=False)
    out = nc.declare_dram_parameter("out", [PB, B, H], BF16, isOutput=True)

    with tile.TileContext(nc) as tc:
        with (
            tc.tile_pool(name="const", bufs=1) as cp,
            tc.tile_pool(name="xchunk", bufs=1) as xp,
            tc.tile_pool(name="hts", bufs=1) as hp,
            tc.tile_pool(name="tz", bufs=6) as tzp,
            tc.tile_pool(name="small", bufs=1) as sp,
            tc.tile_pool(name="sc", bufs=2) as scp,
            tc.tile_pool(name="outp", bufs=4) as op_,
            tc.tile_pool(name="zps", bufs=4, space=bass.MemorySpace.PSUM) as zp,
            tc.tile_pool(name="sps", bufs=2, space=bass.MemorySpace.PSUM) as spp,
            tc.tile_pool(name="aux", bufs=2, space=bass.MemorySpace.PSUM) as ap,
        ):
            # ---- input DMAs: few, large; sync queue in consumption order ----
            blobA_sb = cp.tile([128, 64], mybir.dt.uint8)
            nc.sync.dma_start(blobA_sb[:], blobA[:])
            bh_sb = blobA_sb[:, 0:32].bitcast(F32)
            u8_sb = blobA_sb[:, 32:64].bitcast(FP8).rearrange(
                "p (j q) -> p j q", j=2
            )
            blobB_sb = cp.tile([4, 520], mybir.dt.uint8)
            nc.sync.dma_start(blobB_sb[:], blobB[:])
            selg_sb = blobB_sb[:, 0:512].bitcast(BF16).rearrange(
                "g (q m) -> g q m", q=2
            )
            id_sb = blobB_sb[:, 512:520].bitcast(BF16)
            blobC_sb = cp.tile([1, 4352], mybir.dt.uint8)
            nc.sync.dma_start(blobC_sb[:], blobC[:])
            bpx_sb = blobC_sb[:, 0:4096].bitcast(BF16)
            ones_sb = blobC_sb[:, 4096:4224].bitcast(BF16)

            wm_sb = []

            def _load_wm(m):
                wm = cp.tile([128, KT2, 2, 128], FP8, name=f"wm{m}")
                nc.sync.dma_start(
                    wm[:], whQ8[m].rearrange("p (kt j o) -> p kt j o", j=2, o=128)
                )
                return wm

            xc_sb = []

            def _load_xc(b, split=False):
                xc = xp.tile([128, KT2, 2, T], FP8, name=f"xc{b}")
                halves = ((0, 2), (2, 4)) if split else ((0, 4),)
                for lo, hi in halves:
                    nc.sync.dma_start(
                        xc[:, lo:hi],
                        xQ8[b].rearrange("p (kt j n) -> p kt j n", j=2, n=T)[
                            :, lo:hi
                        ],
                    )
                return xc

            ht_sb = [None] * PB

            wm_sb.append(_load_wm(0))
            xc_sb.append(_load_xc(0, split=True))
            xc_sb.append(_load_xc(1))
            wmrest = cp.tile([128, MT - 1, KT2, 2, 128], FP8)
            nc.sync.dma_start(
                wmrest[:],
                whQ8[1:].rearrange("m p (kt j o) -> p m kt j o", j=2, o=128),
            )
            for m in range(1, MT):
                wm_sb.append(wmrest[:, m - 1])
            for b in (2, 3):
                xc_sb.append(_load_xc(b))
            wxh_sb = cp.tile([128, KT, H], BF16)
            nc.sync.dma_start(wxh_sb[:], wxh[:].rearrange("(kt p) n -> p kt n", p=128))
            hl_sb = cp.tile([128, KT, 2 * B], BF16)
            nc.sync.dma_start(hl_sb[:], hl2[:].rearrange("(kt p) j -> p kt j", p=128))
            hlh_sb = hl_sb[:, :, :B]
            hll_sb = hl_sb[:, :, B:]
            for b in (4, 5):
                xc_sb.append(_load_xc(b))
            htpair = []

            def _load_htpair(hp2):
                htp = hp.tile([128, 2, TT2, 2, H], FP8, name=f"htp{hp2}")
                nc.sync.dma_start(
                    htp[:],
                    ht8[2 * hp2 : 2 * hp2 + 2].rearrange(
                        "b p (tt j h) -> p b tt j h", j=2, h=H
                    ),
                )
                htpair.append(htp)
                ht_sb[2 * hp2] = htp[:, 0]
                ht_sb[2 * hp2 + 1] = htp[:, 1]

            _load_htpair(0)
            _load_htpair(1)
            for b in (6, 7):
                xc_sb.append(_load_xc(b))
            _load_htpair(2)
            _load_htpair(3)
            wpT_sb = cp.tile([128, KT2, 2, H], FP8)
            nc.sync.dma_start(
                wpT_sb[:], wpT8[:].rearrange("p (kt j n) -> p kt j n", j=2, n=H)
            )

            am8 = [None] * len(GROUPS)
            esum_g = [None] * len(GROUPS)
            einv_g = [None] * len(GROUPS)
            x2_sb = sp.tile([128, H], F32)
            s_ps = [None, None]

            def softmax_alpha(b, g, bl):
                """exp(scores) -> e_b + per-group esum; the deferred step
                transposes RAW e values into am8 (dep only on the EXP), and
                1/esum is applied later as a per-row scale on rflat, so the
                PE never waits on the softmax normalization chain."""
                e_b = scp.tile([1, T], BF16, tag="eb")
                nc.scalar.activation(
                    e_b[:1], s_ps[b % 2][:1], EXP, scale=1.0 / USCALE,
                    accum_out=esum_g[g][:1, bl : bl + 1],
                )

                def aT_step():
                    # t = (tt2*2 + jj)*128 + p: chunk c -> col bl of block bl
                    for c in range(4):
                        tp_ps = ap.tile([128, 1], BF16, tag="aux", name="tp")
                        nc.tensor.transpose(
                            tp_ps[:, :1],
                            e_b[:1, c * 128 : (c + 1) * 128],
                            id_sb[:1, :1],
                        )
                        nc.vector.tensor_scalar_mul(
                            am8[g][:, bl, c // 2, c % 2, bl : bl + 1],
                            tp_ps[:, :1],
                            ESC,
                        )

                return aT_step

            def emit_x2(hc):
                """x2 = hlast @ W_x.T + (b_p + b_x), bf16 hi@hi + lo@hi."""
                if True:
                    x_ps = ap.tile([B, 512], F32, tag="aux", name=f"x{hc}")
                    n = 0
                    nmm = 2 * KT + 2
                    for lh in (hlh_sb, hll_sb):
                        for kt in range(KT):
                            nc.tensor.matmul(
                                x_ps[:],
                                lh[:, kt, :],
                                wxh_sb[:, kt, hc * 512 : (hc + 1) * 512],
                                start=(n == 0),
                                stop=(n == nmm - 1),
                            )
                            n += 1
                    for row in range(2):
                        nc.tensor.matmul(
                            x_ps[:],
                            ones_sb[:1, :],
                            bpx_sb[:1, row * H + hc * 512 : row * H + (hc + 1) * 512],
                            start=(n == 0),
                            stop=(n == nmm - 1),
                        )
                        n += 1
                    nc.vector.tensor_scalar_mul(
                        x2_sb[:B, hc * 512 : (hc + 1) * 512], x_ps[:], 1.0
                    )
                    nc.vector.tensor_scalar_mul(
                        x2_sb[B:, hc * 512 : (hc + 1) * 512], x_ps[:], 1.0
                    )

            def boundary_steps(g):
                """r-burst/rT/p/G for group g as steps interleavable into
                the next wave's m-loop."""
                start, size = GROUPS[g]
                st = {}

                def emit_r(hc):
                    if hc == 0:
                        # esum [1,size] -> bf16 -> [size,1] -> einv for rflat
                        es16 = scp.tile([1, 4], BF16, tag="es16")
                        nc.vector.tensor_scalar_mul(
                            es16[:1, :size], esum_g[g][:1, :size], 1.0
                        )
                        et_ps = ap.tile([4, 1], BF16, tag="aux", name="et")
                        nc.tensor.transpose(
                            et_ps[:size, :1], es16[:1, :size], id_sb[:1, :1]
                        )
                        einv_g[g] = sp.tile(
                            [4, 1], F32, tag=f"einv{g}", name=f"einv{g}"
                        )
                        nc.vector.reciprocal(einv_g[g][:size], et_ps[:size, :1])
                    r_ps = ap.tile([4, 512], F32, tag="aux", name=f"r{g}{hc}")
                    n = 0
                    for bl in range(size):
                        for tt in range(TT2):
                            nc.tensor.matmul(
                                r_ps[:size],
                                am8[g][:, bl, tt, :, :size],
                                ht_sb[start + bl][:, tt, :, hc * 512 : (hc + 1) * 512],
                                start=(n == 0),
                                stop=(n == 2 * size - 1),
                                perf_mode=DR,
                            )
                            n += 1
                    rflat = sp.tile([4, 512], BF16, tag="rf", name="rflat", bufs=4)
                    nc.vector.tensor_scalar(
                        rflat[:size],
                        r_ps[:size],
                        einv_g[g][:size, :1],
                        RS / ESC,
                        mybir.AluOpType.mult,
                        mybir.AluOpType.mult,
                    )
                    st[f"rfl{hc}"] = rflat

                def emit_rT():
                    # rT: h = (kt2*2+j)*128 + p = (hc*4 + ktl)*128 + p
                    rT8 = sp.tile(
                        [128, KT2, 2, 16], FP8, tag="rT", name=f"rT{g}", bufs=2
                    )
                    tps = []
                    for hc in range(2):
                        for ktl in range(4):
                            tp_ps = ap.tile([128, 4], BF16, tag="aux", name="tpr")
                            nc.tensor.transpose(
                                tp_ps[:, :size],
                                st[f"rfl{hc}"][:size, ktl * 128 : (ktl + 1) * 128],
                                id_sb[:size, :size],
                            )
                            tps.append((hc * 4 + ktl, tp_ps))
                    for kk, tp_ps in tps:
                        nc.vector.tensor_scalar_mul(
                            rT8[:, kk // 2, kk % 2, :size], tp_ps[:, :size], 1.0
                        )
                    st["rT8"] = rT8

                def emit_p():
                    p_sb = sp.tile([4, H], BF16, tag="psb", name=f"p{g}", bufs=2)
                    for hc in range(2):
                        p_ps = ap.tile([4, 512], F32, tag="aux", name=f"pp{g}{hc}")
                        for kt in range(KT2):
                            nc.tensor.matmul(
                                p_ps[:size],
                                st["rT8"][:, kt, :, :size],
                                wpT_sb[:, kt, :, hc * 512 : (hc + 1) * 512],
                                start=(kt == 0),
                                stop=(kt == KT2 - 1),
                                perf_mode=DR,
                            )
                        nc.vector.tensor_scalar_mul(
                            p_sb[:size, hc * 512 : (hc + 1) * 512],
                            p_ps[:size],
                            1.0 / (RS * WPS),
                        )
                    st["p_sb"] = p_sb

                def emit_G(q):
                    for hc in range(2):
                        o_ps = ap.tile([128, 512], F32, tag="aux", name=f"o{g}{q}{hc}")
                        nc.tensor.matmul(
                            o_ps[:],
                            selg_sb[:size, q, :],
                            st["p_sb"][:size, hc * 512 : (hc + 1) * 512],
                            start=True,
                            stop=True,
                        )
                        o_sb = op_.tile([128, 512], F32, tag="oadd")
                        nc.vector.tensor_tensor(
                            o_sb[:],
                            o_ps[:],
                            x2_sb[:, hc * 512 : (hc + 1) * 512],
                            mybir.AluOpType.add,
                        )
                        o2 = op_.tile([128, 512], BF16, tag="otanh")
                        nc.scalar.activation(o2[:], o_sb[:], TANH)
                        i0 = start + 2 * q
                        nc.scalar.dma_start(
                            out[i0 : i0 + 2, :, hc * 512 : (hc + 1) * 512].rearrange(
                                "i j h -> (i j) h"
                            ),
                            o2[:],
                        )

                steps = [lambda: emit_r(0), lambda: emit_r(1), emit_rT, emit_p]
                for q in range(size // 2):
                    steps.append(lambda q=q: emit_G(q))
                return steps

            # ---- main loop: waves of 2 batches; boundary/x2 steps are
            #      interleaved one per m-iteration to keep the PE dense ----
            pending = []
            for w in range(PB // 2):
                b0 = 2 * w
                g = next(i for i, (s, n) in enumerate(GROUPS) if s <= b0 < s + n)
                gstart, gsize = GROUPS[g]
                if b0 == gstart:
                    am = sp.tile(
                        [128, 4, TT2, 2, 16], FP8, tag=f"am{g}", name=f"am8_{g}"
                    )  # [p, bl, tt2, j, col16]: j-stride 16 for DR ldweights
                    nc.vector.memset(am[:], 0.0)
                    am8[g] = am
                    esum_g[g] = sp.tile([1, 4], F32, tag=f"esum{g}", name=f"es{g}")
                s_ps[0] = spp.tile([1, T], F32, tag="s", name=f"s{b0}")
                s_ps[1] = spp.tile([1, T], F32, tag="s", name=f"s{b0 + 1}")
                tz8 = [None, None]
                for m in range(MT):
                    if pending:
                        pending.pop(0)()
                    z_ps = [
                        zp.tile([128, T], F32, tag="z", name=f"z{b0 + b2}_{m}")
                        for b2 in range(2)
                    ]
                    for kt in range(KT2):
                        for b2 in range(2):
                            nc.tensor.matmul(
                                z_ps[b2][:],
                                wm_sb[m][:, kt, :, :],
                                xc_sb[b0 + b2][:, kt, :, :],
                                start=(kt == 0),
                                stop=(kt == KT2 - 1),
                                perf_mode=DR,
                            )
                    q, jj = m // 2, m % 2
                    for b2 in range(2):
                        if jj == 0:
                            tz8[b2] = tzp.tile(
                                [128, 2, T], FP8, tag="tz8", name=f"tz{b0+b2}_{q}"
                            )
                        nc.scalar.activation(
                            tz8[b2][:, jj, :],
                            z_ps[b2][:],
                            TANH,
                            bias=bh_sb[:, m : m + 1],
                            scale=1.0 / WSCALE,
                        )
                    if jj == 1:
                        for b2 in range(2):
                            nc.tensor.matmul(
                                s_ps[b2][:1, :],
                                u8_sb[:, :, q : q + 1],
                                tz8[b2][:],
                                start=(q == 0),
                                stop=(q == KT2 - 1),
                                perf_mode=DR,
                            )
                for step in pending:
                    step()
                pending = []
                for b2 in range(2):
                    b = b0 + b2
                    pending.append(softmax_alpha(b, g, b - gstart))
                if w == 0:
                    pending += [lambda: emit_x2(0), lambda: emit_x2(1)]
                if b0 + 2 == gstart + gsize:
                    pending.extend(boundary_steps(g))
            for step in pending:
                step()
    _split_excess_waits(nc)
    return nc


def _split_excess_waits(nc: bass.Bass, max_waits: int = 1) -> None:
    """Walrus's per-instruction sync-wait slots are limited; move excess
    on_wait entries onto wait-only NoOps inserted just before the
    instruction (same engine, so ordering is preserved)."""
    for fn in nc.m.functions:
        for blk in fn.blocks:
            new = []
            for inst in blk.instructions:
                si = inst.sync_info
                waits = list(si.on_wait) if si is not None and si.on_wait else []
                if len(waits) > max_waits:
                    extra, keep = waits[:-max_waits], waits[-max_waits:]
                    for ci in range(0, len(extra), max_waits):
                        nop = mybir.InstNoOp(
                            name=f"{inst.name}-wsplit{ci}", ins=[], outs=[]
                        )
                        nop.engine = inst.engine
                        nop.sync_info = mybir.SyncInfo(
                            on_wait=extra[ci : ci + max_waits], on_update=[]
                        )
                        new.append(nop)
                    inst.sync_info = mybir.SyncInfo(
                        on_wait=keep, on_update=list(si.on_update or [])
                    )
                new.append(inst)
            blk.instructions[:] = new


def _split_bf16(a: np.ndarray) -> tuple[np.ndarray, np.ndarray]:
    hi = a.astype(BF16_NP)
    lo = (a - hi.astype(np.float32)).astype(BF16_NP)
    return hi, lo


def _pad_u8(w_w: np.ndarray) -> np.ndarray:
    u = np.zeros((128, 2, 16), np.float32)
    u[:, :, :KT2] = (w_w[0, :H] * USCALE).reshape(KT2, 2, 128).transpose(2, 1, 0)
    return np.ascontiguousarray(u.reshape(128, 2 * 16)).astype(FP8_NP)


def _host_prep(inputs: dict) -> list[dict]:
    hidden = np.asarray(inputs["hidden"], np.float32)
    W_h = np.asarray(inputs["W_h"], np.float32)
    b_h = np.asarray(inputs["b_h"], np.float32)
    w_w = np.asarray(inputs["w_w"], np.float32)
    W_p = np.asarray(inputs["W_p"], np.float32)
    b_p = np.asarray(inputs["b_p"], np.float32)
    W_x = np.asarray(inputs["W_x"], np.float32)
    b_x = np.asarray(inputs["b_x"], np.float32)

    # G selector: row i = 2q + m//64 (q=0 block alone serves 2-row groups)
    selgm = np.zeros((4, 2, 128), np.float32)
    for q in range(2):
        for m in range(128):
            selgm[2 * q + m // 64, q, m] = 1.0

    wxT = np.ascontiguousarray(W_x.T)
    wx_hi, _ = _split_bf16(wxT)
    hlT = np.ascontiguousarray(hidden[:, -1, :].T)
    hl_hi, hl_lo = _split_bf16(hlT)
    bpx_hi, bpx_lo = _split_bf16((b_p + b_x).reshape(1, H))

    bhm = np.ascontiguousarray(b_h.reshape(MT, 128).T, np.float32)
    blobA = np.concatenate(
        [bhm.view(np.uint8).reshape(128, 32),
         _pad_u8(w_w).view(np.uint8).reshape(128, 32)], axis=1
    )
    selg_b = selgm.reshape(4, 2 * 128).astype(BF16_NP)
    ident_b = np.eye(4, dtype=np.float32).astype(BF16_NP)
    blobB = np.concatenate(
        [selg_b.view(np.uint8).reshape(4, 512),
         ident_b.view(np.uint8).reshape(4, 8)], axis=1
    )
    bpx_b = np.concatenate([bpx_hi, bpx_lo], axis=1)
    ones_b = np.ones((1, B), BF16_NP)
    blobC = np.concatenate(
        [bpx_b.view(np.uint8).reshape(1, 4096),
         ones_b.view(np.uint8).reshape(1, 128),
         np.zeros((1, 128), np.uint8)], axis=1
    )
    shared = {
        "whQ8": np.ascontiguousarray(
            (W_h.T * WSCALE)
            .reshape(KT2, 128, 2, MT, 128)
            .transpose(3, 1, 0, 2, 4)
            .reshape(MT, 128, KT2 * 2 * 128)
        ).astype(FP8_NP),
        "blobA": blobA,
        "blobB": blobB,
        "blobC": blobC,
        # wpT8[p, kt2, j, n] = W_p.T[(kt2*2+j)*128 + p, n] * WPS
        "wpT8": np.ascontiguousarray(
            (W_p.T * WPS).reshape(KT2, 2, 128, H).transpose(2, 0, 1, 3).reshape(
                128, KT2 * 2 * H
            )
        ).astype(FP8_NP),
        "wxT_hi": wx_hi,
        "hl2": np.ascontiguousarray(np.concatenate([hl_hi, hl_lo], axis=1)),
    }

    in_maps = []
    for c in range(NCORES):
        slab = hidden[c * PB : (c + 1) * PB]          # [PB, T, H]
        m = dict(shared)
        m["xQ8"] = np.ascontiguousarray(
            slab.reshape(PB, T, KT2, 128, 2)
            .transpose(0, 3, 2, 4, 1)
            .reshape(PB, 128, KT2 * 2 * T)
        ).astype(FP8_NP)
        # ht8[b][p, tt2, j, h] = hidden[b, t = (tt2*2+j)*128 + p, h]
        m["ht8"] = np.ascontiguousarray(
            slab.reshape(PB, TT2, 2, 128, H)
            .transpose(0, 3, 1, 2, 4)
            .reshape(PB, 128, TT2 * 2 * H)
        ).astype(FP8_NP)
        in_maps.append(m)
    return in_maps


def _ensure_ntff_hook() -> None:
    """The agent image's antenv lacks axon_hooks; register a shim module
    wired to the libaxon NTFF profile hook so trace=True works."""
    try:
        from antenv.axon_hooks import get_axon_ntff_profile_hook  # noqa: F401
        return
    except ImportError:
        pass
    import types
    import antenv
    from trn_agent_boot.trn_boot import _ntff_profile_via_ctypes

    mod = types.ModuleType("antenv.axon_hooks")
    holder = {"hook": _ntff_profile_via_ctypes("/opt/axon/libaxon_pjrt.so")}
    mod.get_axon_ntff_profile_hook = lambda: holder["hook"]
    mod.set_axon_ntff_profile_hook = lambda h: holder.__setitem__("hook", h)
    sys.modules["antenv.axon_hooks"] = mod
    antenv.axon_hooks = mod


def run(inputs: dict, trace: bool = False, **kw):
    if trace:
        _ensure_ntff_hook()
    if "nc" not in _CACHE:
        _CACHE["nc"] = _build_nc()
    nc = _CACHE["nc"]
    in_maps = _host_prep(inputs)
    res = run_bass_kernel_spmd(nc, in_maps, list(range(NCORES)), trace=trace, **kw)
    out = np.empty((B, B, H), np.float32)
    for c in range(NCORES):
        out[c * PB : (c + 1) * PB] = np.asarray(res.results[c]["out"], np.float32)
    return out, res


def kernel(**inputs) -> np.ndarray:
    out, _ = run(inputs)
    return out


# revision 18
# speedup vs baseline: 1.1901x; 1.1827x over previous
"""TRN2 Bass kernel for nn_Attention_76802605187492.

Math (B=64, T=512, H=1024, A=300):
  The aspect branch only adds a per-batch constant to the attention
  scores, which softmax cancels, so it does not affect the output.
  Per batch b:
    scores[t] = u . tanh(W_h hidden[b,t] + b_h)      u = w_w[0, :H]
    alpha     = softmax_t(scores)
    r         = sum_t alpha[t] hidden[b,t]
    p_b       = r @ W_p.T
    x_j       = hidden[j,-1] @ W_x.T                  (all j)
    out[b,j]  = tanh(p_b + x_j + (b_p + b_x))         -> [B, B, H]

Sharding: data-parallel over batch across 8 cores (8 batches each).

v4 design (vs 139us baseline):
  - big matmul, scores, r and p all in fp8 DoubleRow (2 k-rows/cycle).
    Scores pair m-tiles on the j dim (tanh writes fp8 tz directly);
    r contracts t via an fp8 t-layout upload ht8 (t = (tt2*2+j)*128+p),
    alpha transposed into masked am8 columns, 4 chunks per batch.
    DR ldweights requires the j-plane stride to be >= 16 elements
    (u8/rT8/am8 padded accordingly).
  - batches processed in waves of 2. The SP engine issues DMA
    descriptors at ~1us each and the Tile scheduler hoists dep-free
    issues, so ALL inputs ride the single sync queue as ~19 large DMAs
    in consumption-priority order (the queue executes descriptors
    FIFO); outputs go on the scalar queue.
  - boundary work (r-burst/rT/p/G) is interleaved one step per
    m-iteration into the NEXT wave's matmul stream so the PE never
    idles long enough for the HAM clock-gate to throttle it.
  - x2 (hlast @ W_x.T, bf16 hi@hi + lo@hi; bias rides via k=1 ones
    matmuls) is issued right after wave 0 where the PE is DMA-starved.
  - output pipelined in 3 groups (batches 0-3, 4-5, 6-7): each group's
    r-burst/rT/p/G/out-DMA runs right after its last wave, so only the
    last 2 batches' output (0.26MB) drains at the end.
  - softmax exp uses accum_out to fuse the row-sum.
  - psum: z x4 (2KB slots, also used by boundary transposes), s x2,
    aux x2 = exactly 8 banks.
"""

import os
import sys

sys.path.insert(0, "/opt/trn_rl_repo")
sys.path.insert(0, "/opt/trn_rl_repo/concourse")

import numpy as np
import ml_dtypes

import concourse.bass as bass
import concourse.mybir as mybir
from concourse import tile
from concourse.bass_utils import run_bass_kernel_spmd

F32 = mybir.dt.float32
BF16 = mybir.dt.bfloat16
BF16_NP = ml_dtypes.bfloat16
TANH = mybir.ActivationFunctionType.Tanh
EXP = mybir.ActivationFunctionType.Exp
FP8 = mybir.dt.float8e4
FP8_NP = ml_dtypes.float8_e4m3
DR = mybir.MatmulPerfMode.DoubleRow
WSCALE = 16.0     # W_h pre-scale into fp8 range
USCALE = 128.0    # u pre-scale into fp8 range
ASCALE = 128.0    # alpha pre-scale (max alpha=1 -> 128 < 240 fp8 max)
RS = 32.0         # r pre-scale into fp8 range
ESC = 8.0         # exp pre-scale into fp8 range (e in (0, ~13))
WPS = 16.0        # W_p pre-scale into fp8 range

B, T, H = 64, 512, 1024
NCORES = 8
PB = B // NCORES          # batches per core = 8
KT = H // 128             # 8 k-tiles over h_in
MT = H // 128             # 8 m-tiles over h_out
KT2 = H // 256            # 4 double-row k-tiles
TT2 = T // 256            # 2 double-row t-tiles for r
GROUPS = [(0, 4), (4, 2), (6, 2)]   # (first batch, size) output groups

_CACHE: dict = {}


def _build_nc() -> bass.Bass:
    nc = bass.Bass()

    xQ8 = nc.declare_dram_parameter("xQ8", [PB, 128, KT2 * 2 * T], FP8, isOutput=False)
    whQ8 = nc.declare_dram_parameter(
        "whQ8", [MT, 128, KT2 * 2 * 128], FP8, isOutput=False
    )
    ht8 = nc.declare_dram_parameter("ht8", [PB, 128, TT2 * 2 * H], FP8, isOutput=False)
    blobA = nc.declare_dram_parameter("blobA", [128, 64], mybir.dt.uint8,
                                      isOutput=False)
    blobB = nc.declare_dram_parameter("blobB", [4, 520], mybir.dt.uint8,
                                      isOutput=False)
    blobC = nc.declare_dram_parameter("blobC", [1, 4352], mybir.dt.uint8,
                                      isOutput=False)
    wpT8 = nc.declare_dram_parameter("wpT8", [128, KT2 * 2 * H], FP8, isOutput=False)
    wxh = nc.declare_dram_parameter("wxT_hi", [H, H], BF16, isOutput=False)
    hl2 = nc.declare_dram_parameter("hl2", [H, 2 * B], BF16, isOutput=False)
    out = nc.declare_dram_parameter("out", [PB, B, H], BF16, isOutput=True)

    with tile.TileContext(nc) as tc:
        with (
            tc.tile_pool(name="const", bufs=1) as cp,
            tc.tile_pool(name="xchunk", bufs=1) as xp,
            tc.tile_pool(name="hts", bufs=1) as hp,
            tc.tile_pool(name="tz", bufs=6) as tzp,
            tc.tile_pool(name="small", bufs=1) as sp,
            tc.tile_pool(name="sc", bufs=2) as scp,
            tc.tile_pool(name="outp", bufs=4) as op_,
            tc.tile_pool(name="zps", bufs=4, space=bass.MemorySpace.PSUM) as zp,
            tc.tile_pool(name="sps", bufs=2, space=bass.MemorySpace.PSUM) as spp,
            tc.tile_pool(name="aux", bufs=2, space=bass.MemorySpace.PSUM) as ap,
        ):
            # ---- input DMAs: few, large; sync queue in consumption order ----
            blobA_sb = cp.tile([128, 64], mybir.dt.uint8)
            nc.sync.dma_start(blobA_sb[:], blobA[:])
            bh_sb = blobA_sb[:, 0:32].bitcast(F32)
            u8_sb = blobA_sb[:, 32:64].bitcast(FP8).rearrange(
                "p (j q) -> p j q", j=2
            )
            blobB_sb = cp.tile([4, 520], mybir.dt.uint8)
            nc.sync.dma_start(blobB_sb[:], blobB[:])
            selg_sb = blobB_sb[:, 0:512].bitcast(BF16).rearrange(
                "g (q m) -> g q m", q=2
            )
            id_sb = blobB_sb[:, 512:520].bitcast(BF16)
            blobC_sb = cp.tile([1, 4352], mybir.dt.uint8)
            nc.sync.dma_start(blobC_sb[:], blobC[:])
            bpx_sb = blobC_sb[:, 0:4096].bitcast(BF16)
            ones_sb = blobC_sb[:, 4096:4224].bitcast(BF16)

            wm_sb = []

            def _load_wm(m):
                wm = cp.tile([128, KT2, 2, 128], FP8, name=f"wm{m}")
                nc.sync.dma_start(
                    wm[:], whQ8[m].rearrange("p (kt j o) -> p kt j o", j=2, o=128)
                )
                return wm

            xc_sb = []

            def _load_xc(b, split=False):
                xc = xp.tile([128, KT2, 2, T], FP8, name=f"xc{b}")
                halves = ((0, 2), (2, 4)) if split else ((0, 4),)
                for lo, hi in halves:
                    nc.sync.dma_start(
                        xc[:, lo:hi],
                        xQ8[b].rearrange("p (kt j n) -> p kt j n", j=2, n=T)[
                            :, lo:hi
                        ],
                    )
                return xc

            ht_sb = [None] * PB

            wm_sb.append(_load_wm(0))
            xc_sb.append(_load_xc(0, split=True))
            xc_sb.append(_load_xc(1))
            wmrest = cp.tile([128, MT - 1, KT2, 2, 128], FP8)
            nc.sync.dma_start(
                wmrest[:],
                whQ8[1:].rearrange("m p (kt j o) -> p m kt j o", j=2, o=128),
            )
            for m in range(1, MT):
                wm_sb.append(wmrest[:, m - 1])
            for b in (2, 3):
                xc_sb.append(_load_xc(b))
            wxh_sb = cp.tile([128, KT, H], BF16)
            nc.sync.dma_start(wxh_sb[:], wxh[:].rearrange("(kt p) n -> p kt n", p=128))
            hl_sb = cp.tile([128, KT, 2 * B], BF16)
            nc.sync.dma_start(hl_sb[:], hl2[:].rearrange("(kt p) j -> p kt j", p=128))
            hlh_sb = hl_sb[:, :, :B]
            hll_sb = hl_sb[:, :, B:]
            for b in (4, 5):
                xc_sb.append(_load_xc(b))
            htpair = []

            def _load_htpair(hp2):
                htp = hp.tile([128, 2, TT2, 2, H], FP8, name=f"htp{hp2}")
                nc.sync.dma_start(
                    htp[:],
                    ht8[2 * hp2 : 2 * hp2 + 2].rearrange(
                        "b p (tt j h) -> p b tt j h", j=2, h=H
                    ),
                )
                htpair.append(htp)
                ht_sb[2 * hp2] = htp[:, 0]
                ht_sb[2 * hp2 + 1] = htp[:, 1]

            _load_htpair(0)
            _load_htpair(1)
            for b in (6, 7):
                xc_sb.append(_load_xc(b))
            _load_htpair(2)
            _load_htpair(3)
            wpT_sb = cp.tile([128, KT2, 2, H], FP8)
            nc.sync.dma_start(
                wpT_sb[:], wpT8[:].rearrange("p (kt j n) -> p kt j n", j=2, n=H)
            )

            am8 = [None] * len(GROUPS)
            esum_g = [None] * len(GROUPS)
            einv_g = [None] * len(GROUPS)
            x2_sb = sp.tile([128, H], F32)
            s_ps = [None, None]

            def softmax_alpha(b, g, bl):
                """exp(scores) -> e_b + per-group esum, then transpose RAW e
                values into am8 (dep only on the EXP, so the wave-end stall
                is just the EXP latency); 1/esum is applied later as a
                per-row scale on rflat."""
                e_b = scp.tile([1, T], BF16, tag="eb")
                nc.scalar.activation(
                    e_b[:1], s_ps[b % 2][:1], EXP, scale=1.0 / USCALE,
                    accum_out=esum_g[g][:1, bl : bl + 1],
                )
                # t = (tt2*2 + jj)*128 + p: chunk c -> column bl of block bl
                for c in range(4):
                    tp_ps = zp.tile([128, 1], BF16, tag="z", name="tp")
                    nc.tensor.transpose(
                        tp_ps[:, :1], e_b[:1, c * 128 : (c + 1) * 128], id_sb[:1, :1]
                    )
                    nc.vector.tensor_scalar_mul(
                        am8[g][:, bl, c // 2, c % 2, bl : bl + 1], tp_ps[:, :1], ESC
                    )

            def emit_x2(hc):
                """x2 = hlast @ W_x.T + (b_p + b_x), bf16 hi@hi + lo@hi."""
                if True:
                    x_ps = ap.tile([B, 512], F32, tag="aux", name=f"x{hc}")
                    n = 0
                    nmm = 2 * KT + 2
                    for lh in (hlh_sb, hll_sb):
                        for kt in range(KT):
                            nc.tensor.matmul(
                                x_ps[:],
                                lh[:, kt, :],
                                wxh_sb[:, kt, hc * 512 : (hc + 1) * 512],
                                start=(n == 0),
                                stop=(n == nmm - 1),
                            )
                            n += 1
                    for row in range(2):
                        nc.tensor.matmul(
                            x_ps[:],
                            ones_sb[:1, :],
                            bpx_sb[:1, row * H + hc * 512 : row * H + (hc + 1) * 512],
                            start=(n == 0),
                            stop=(n == nmm - 1),
                        )
                        n += 1
                    nc.vector.tensor_scalar_mul(
                        x2_sb[:B, hc * 512 : (hc + 1) * 512], x_ps[:], 1.0
                    )
                    nc.vector.tensor_scalar_mul(
                        x2_sb[B:, hc * 512 : (hc + 1) * 512], x_ps[:], 1.0
                    )

            def boundary_steps(g):
                """r-burst/rT/p/G for group g as steps interleavable into
                the next wave's m-loop."""
                start, size = GROUPS[g]
                st = {}

                def emit_r(hc):
                    if hc == 0:
                        # esum [1,size] -> bf16 -> [size,1] -> einv for rflat
                        es16 = scp.tile([1, 4], BF16, tag="es16")
                        nc.vector.tensor_scalar_mul(
                            es16[:1, :size], esum_g[g][:1, :size], 1.0
                        )
                        et_ps = zp.tile([4, 1], BF16, tag="z", name="et")
                        nc.tensor.transpose(
                            et_ps[:size, :1], es16[:1, :size], id_sb[:1, :1]
                        )
                        einv_g[g] = sp.tile(
                            [4, 1], F32, tag=f"einv{g}", name=f"einv{g}"
                        )
                        nc.vector.reciprocal(einv_g[g][:size], et_ps[:size, :1])
                    r_ps = ap.tile([4, 512], F32, tag="aux", name=f"r{g}{hc}")
                    n = 0
                    for bl in range(size):
                        for tt in range(TT2):
                            nc.tensor.matmul(
                                r_ps[:size],
                                am8[g][:, bl, tt, :, :size],
                                ht_sb[start + bl][:, tt, :, hc * 512 : (hc + 1) * 512],
                                start=(n == 0),
                                stop=(n == 2 * size - 1),
                                perf_mode=DR,
                            )
                            n += 1
                    rflat = sp.tile([4, 512], BF16, tag="rf", name="rflat", bufs=4)
                    nc.vector.tensor_scalar(
                        rflat[:size],
                        r_ps[:size],
                        einv_g[g][:size, :1],
                        RS / ESC,
                        mybir.AluOpType.mult,
                        mybir.AluOpType.mult,
                    )
                    st[f"rfl{hc}"] = rflat

                def emit_rT():
                    # rT: h = (kt2*2+j)*128 + p = (hc*4 + ktl)*128 + p
                    rT8 = sp.tile(
                        [128, KT2, 2, 16], FP8, tag="rT", name=f"rT{g}", bufs=2
                    )
                    tps = []
                    for hc in range(2):
                        for ktl in range(4):
                            tp_ps = zp.tile([128, 4], BF16, tag="z", name="tpr")
                            nc.tensor.transpose(
                                tp_ps[:, :size],
                                st[f"rfl{hc}"][:size, ktl * 128 : (ktl + 1) * 128],
                                id_sb[:size, :size],
                            )
                            tps.append((hc * 4 + ktl, tp_ps))
                    for kk, tp_ps in tps:
                        nc.vector.tensor_scalar_mul(
                            rT8[:, kk // 2, kk % 2, :size], tp_ps[:, :size], 1.0
                        )
                    st["rT8"] = rT8

                def emit_p():
                    p_sb = sp.tile([4, H], BF16, tag="psb", name=f"p{g}", bufs=2)
                    for hc in range(2):
                        p_ps = ap.tile([4, 512], F32, tag="aux", name=f"pp{g}{hc}")
                        for kt in range(KT2):
                            nc.tensor.matmul(
                                p_ps[:size],
                                st["rT8"][:, kt, :, :size],
                                wpT_sb[:, kt, :, hc * 512 : (hc + 1) * 512],
                                start=(kt == 0),
                                stop=(kt == KT2 - 1),
                                perf_mode=DR,
                            )
                        nc.vector.tensor_scalar_mul(
                            p_sb[:size, hc * 512 : (hc + 1) * 512],
                            p_ps[:size],
                            1.0 / (RS * WPS),
                        )
                    st["p_sb"] = p_sb

                def emit_G(q):
                    for hc in range(2):
                        o_ps = ap.tile([128, 512], F32, tag="aux", name=f"o{g}{q}{hc}")
                        nc.tensor.matmul(
                            o_ps[:],
                            selg_sb[:size, q, :],
                            st["p_sb"][:size, hc * 512 : (hc + 1) * 512],
                            start=True,
                            stop=True,
                        )
                        o_sb = op_.tile([128, 512], F32, tag="oadd")
                        nc.vector.tensor_tensor(
                            o_sb[:],
                            o_ps[:],
                            x2_sb[:, hc * 512 : (hc + 1) * 512],
                            mybir.AluOpType.add,
                        )
                        o2 = op_.tile([128, 512], BF16, tag="otanh")
                        nc.scalar.activation(o2[:], o_sb[:], TANH)
                        i0 = start + 2 * q
                        nc.scalar.dma_start(
                            out[i0 : i0 + 2, :, hc * 512 : (hc + 1) * 512].rearrange(
                                "i j h -> (i j) h"
                            ),
                            o2[:],
                        )

                steps = [lambda: emit_r(0), lambda: emit_r(1), emit_rT, emit_p]
                for q in range(size // 2):
                    steps.append(lambda q=q: emit_G(q))
                return steps

            # ---- main loop: waves of 2 batches; boundary/x2 steps are
            #      interleaved one per m-iteration to keep the PE dense ----
            pending = []
            for w in range(PB // 2):
                b0 = 2 * w
                g = next(i for i, (s, n) in enumerate(GROUPS) if s <= b0 < s + n)
                gstart, gsize = GROUPS[g]
                if b0 == gstart:
                    am = sp.tile(
                        [128, 4, TT2, 2, 16], FP8, tag=f"am{g}", name=f"am8_{g}"
                    )  # [p, bl, tt2, j, col16]: j-stride 16 for DR ldweights
                    nc.vector.memset(am[:], 0.0)
                    am8[g] = am
                    esum_g[g] = sp.tile([1, 4], F32, tag=f"esum{g}", name=f"es{g}")
                s_ps[0] = spp.tile([1, T], F32, tag="s", name=f"s{b0}")
                s_ps[1] = spp.tile([1, T], F32, tag="s", name=f"s{b0 + 1}")
                tz8 = [None, None]
                for m in range(MT):
                    if pending and m >= 2:
                        pending.pop(0)()
                    z_ps = [
                        zp.tile([128, T], F32, tag="z", name=f"z{b0 + b2}_{m}")
                        for b2 in range(2)
                    ]
                    for kt in range(KT2):
                        for b2 in range(2):
                            nc.tensor.matmul(
                                z_ps[b2][:],
                                wm_sb[m][:, kt, :, :],
                                xc_sb[b0 + b2][:, kt, :, :],
                                start=(kt == 0),
                                stop=(kt == KT2 - 1),
                                perf_mode=DR,
                            )
                    q, jj = m // 2, m % 2
                    for b2 in range(2):
                        if jj == 0:
                            tz8[b2] = tzp.tile(
                                [128, 2, T], FP8, tag="tz8", name=f"tz{b0+b2}_{q}"
                            )
                        nc.scalar.activation(
                            tz8[b2][:, jj, :],
                            z_ps[b2][:],
                            TANH,
                            bias=bh_sb[:, m : m + 1],
                            scale=1.0 / WSCALE,
                        )
                    if jj == 1:
                        for b2 in range(2):
                            nc.tensor.matmul(
                                s_ps[b2][:1, :],
                                u8_sb[:, :, q : q + 1],
                                tz8[b2][:],
                                start=(q == 0),
                                stop=(q == KT2 - 1),
                                perf_mode=DR,
                            )
                for b2 in range(2):
                    b = b0 + b2
                    softmax_alpha(b, g, b - gstart)
                for step in pending:
                    step()
                pending = []
                if w == 0:
                    pending = [lambda: emit_x2(0), lambda: emit_x2(1)]
                if b0 + 2 == gstart + gsize:
                    pending.extend(boundary_steps(g))
            for step in pending:
                step()
    _split_excess_waits(nc)
    return nc


def _split_excess_waits(nc: bass.Bass, max_waits: int = 1) -> None:
    """Walrus's per-instruction sync-wait slots are limited; move excess
    on_wait entries onto wait-only NoOps inserted just before the
    instruction (same engine, so ordering is preserved)."""
    for fn in nc.m.functions:
        for blk in fn.blocks:
            new = []
            for inst in blk.instructions:
                si = inst.sync_info
                waits = list(si.on_wait) if si is not None and si.on_wait else []
                if len(waits) > max_waits:
                    extra, keep = waits[:-max_waits], waits[-max_waits:]
                    for ci in range(0, len(extra), max_waits):
                        nop = mybir.InstNoOp(
                            name=f"{inst.name}-wsplit{ci}", ins=[], outs=[]
                        )
                        nop.engine = inst.engine
                        nop.sync_info = mybir.SyncInfo(
                            on_wait=extra[ci : ci + max_waits], on_update=[]
                        )
                        new.append(nop)
                    inst.sync_info = mybir.SyncInfo(
                        on_wait=keep, on_update=list(si.on_update or [])
                    )
                new.append(inst)
            blk.instructions[:] = new


def _split_bf16(a: np.ndarray) -> tuple[np.ndarray, np.ndarray]:
    hi = a.astype(BF16_NP)
    lo = (a - hi.astype(np.float32)).astype(BF16_NP)
    return hi, lo


def _pad_u8(w_w: np.ndarray) -> np.ndarray:
    u = np.zeros((128, 2, 16), np.float32)
    u[:, :, :KT2] = (w_w[0, :H] * USCALE).reshape(KT2, 2, 128).transpose(2, 1, 0)
    return np.ascontiguousarray(u.reshape(128, 2 * 16)).astype(FP8_NP)


def _host_prep(inputs: dict) -> list[dict]:
    hidden = np.asarray(inputs["hidden"], np.float32)
    W_h = np.asarray(inputs["W_h"], np.float32)
    b_h = np.asarray(inputs["b_h"], np.float32)
    w_w = np.asarray(inputs["w_w"], np.float32)
    W_p = np.asarray(inputs["W_p"], np.float32)
    b_p = np.asarray(inputs["b_p"], np.float32)
    W_x = np.asarray(inputs["W_x"], np.float32)
    b_x = np.asarray(inputs["b_x"], np.float32)

    # G selector: row i = 2q + m//64 (q=0 block alone serves 2-row groups)
    selgm = np.zeros((4, 2, 128), np.float32)
    for q in range(2):
        for m in range(128):
            selgm[2 * q + m // 64, q, m] = 1.0

    wxT = np.ascontiguousarray(W_x.T)
    wx_hi, _ = _split_bf16(wxT)
    hlT = np.ascontiguousarray(hidden[:, -1, :].T)
    hl_hi, hl_lo = _split_bf16(hlT)
    bpx_hi, bpx_lo = _split_bf16((b_p + b_x).reshape(1, H))

    bhm = np.ascontiguousarray(b_h.reshape(MT, 128).T, np.float32)
    blobA = np.concatenate(
        [bhm.view(np.uint8).reshape(128, 32),
         _pad_u8(w_w).view(np.uint8).reshape(128, 32)], axis=1
    )
    selg_b = selgm.reshape(4, 2 * 128).astype(BF16_NP)
    ident_b = np.eye(4, dtype=np.float32).astype(BF16_NP)
    blobB = np.concatenate(
        [selg_b.view(np.uint8).reshape(4, 512),
         ident_b.view(np.uint8).reshape(4, 8)], axis=1
    )
    bpx_b = np.concatenate([bpx_hi, bpx_lo], axis=1)
    ones_b = np.ones((1, B), BF16_NP)
    blobC = np.concatenate(
        [bpx_b.view(np.uint8).reshape(1, 4096),
         ones_b.view(np.uint8).reshape(1, 128),
         np.zeros((1, 128), np.uint8)], axis=1
    )
    shared = {
        "whQ8": np.ascontiguousarray(
            (W_h.T * WSCALE)
            .reshape(KT2, 128, 2, MT, 128)
            .transpose(3, 1, 0, 2, 4)
            .reshape(MT, 128, KT2 * 2 * 128)
        ).astype(FP8_NP),
        "blobA": blobA,
        "blobB": blobB,
        "blobC": blobC,
        # wpT8[p, kt2, j, n] = W_p.T[(kt2*2+j)*128 + p, n] * WPS
        "wpT8": np.ascontiguousarray(
            (W_p.T * WPS).reshape(KT2, 2, 128, H).transpose(2, 0, 1, 3).reshape(
                128, KT2 * 2 * H
            )
        ).astype(FP8_NP),
        "wxT_hi": wx_hi,
        "hl2": np.ascontiguousarray(np.concatenate([hl_hi, hl_lo], axis=1)),
    }

    in_maps = []
    for c in range(NCORES):
        slab = hidden[c * PB : (c + 1) * PB]          # [PB, T, H]
        m = dict(shared)
        m["xQ8"] = np.ascontiguousarray(
            slab.reshape(PB, T, KT2, 128, 2)
            .transpose(0, 3, 2, 4, 1)
            .reshape(PB, 128, KT2 * 2 * T)
        ).astype(FP8_NP)
        # ht8[b][p, tt2, j, h] = hidden[b, t = (tt2*2+j)*128 + p, h]
        m["ht8"] = np.ascontiguousarray(
            slab.reshape(PB, TT2, 2, 128, H)
            .transpose(0, 3, 1, 2, 4)
            .reshape(PB, 128, TT2 * 2 * H)
        ).astype(FP8_NP)
        in_maps.append(m)
    return in_maps


def _ensure_ntff_hook() -> None:
    """The agent image's antenv lacks axon_hooks; register a shim module
    wired to the libaxon NTFF profile hook so trace=True works."""
    try:
        from antenv.axon_hooks import get_axon_ntff_profile_hook  # noqa: F401
        return
    except ImportError:
        pass
    import types
    import antenv
    from trn_agent_boot.trn_boot import _ntff_profile_via_ctypes

    mod = types.ModuleType("antenv.axon_hooks")
    holder = {"hook": _ntff_profile_via_ctypes("/opt/axon/libaxon_pjrt.so")}
    mod.get_axon_ntff_profile_hook = lambda: holder["hook"]
    mod.set_axon_ntff_profile_hook = lambda h: holder.__setitem__("hook", h)
    sys.modules["antenv.axon_hooks"] = mod
    antenv.axon_hooks = mod


def run(inputs: dict, trace: bool = False, **kw):
    if trace:
        _ensure_ntff_hook()
    if "nc" not in _CACHE:
        _CACHE["nc"] = _build_nc()
    nc = _CACHE["nc"]
    in_maps = _host_prep(inputs)
    res = run_bass_kernel_spmd(nc, in_maps, list(range(NCORES)), trace=trace, **kw)
    out = np.empty((B, B, H), np.float32)
    for c in range(NCORES):
        out[c * PB : (c + 1) * PB] = np.asarray(res.results[c]["out"], np.float32)
    return out, res


def kernel(**inputs) -> np.ndarray:
    out, _ = run(inputs)
    return out


# revision 19
# speedup vs baseline: 1.2173x; 1.0228x over previous
"""TRN2 Bass kernel for nn_Attention_76802605187492.

Math (B=64, T=512, H=1024, A=300):
  The aspect branch only adds a per-batch constant to the attention
  scores, which softmax cancels, so it does not affect the output.
  Per batch b:
    scores[t] = u . tanh(W_h hidden[b,t] + b_h)      u = w_w[0, :H]
    alpha     = softmax_t(scores)
    r         = sum_t alpha[t] hidden[b,t]
    p_b       = r @ W_p.T
    x_j       = hidden[j,-1] @ W_x.T                  (all j)
    out[b,j]  = tanh(p_b + x_j + (b_p + b_x))         -> [B, B, H]

Sharding: data-parallel over batch across 8 cores (8 batches each).

v4 design (vs 139us baseline):
  - big matmul, scores, r and p all in fp8 DoubleRow (2 k-rows/cycle).
    Scores pair m-tiles on the j dim (tanh writes fp8 tz directly);
    r contracts t via an fp8 t-layout upload ht8 (t = (tt2*2+j)*128+p),
    alpha transposed into masked am8 columns, 4 chunks per batch.
    DR ldweights requires the j-plane stride to be >= 16 elements
    (u8/rT8/am8 padded accordingly).
  - batches processed in waves of 2. The SP engine issues DMA
    descriptors at ~1us each and the Tile scheduler hoists dep-free
    issues, so ALL inputs ride the single sync queue as ~19 large DMAs
    in consumption-priority order (the queue executes descriptors
    FIFO); outputs go on the scalar queue.
  - boundary work (r-burst/rT/p/G) is interleaved one step per
    m-iteration into the NEXT wave's matmul stream so the PE never
    idles long enough for the HAM clock-gate to throttle it.
  - x2 (hlast @ W_x.T, bf16 hi@hi + lo@hi; bias rides via k=1 ones
    matmuls) is issued right after wave 0 where the PE is DMA-starved.
  - output pipelined in 3 groups (batches 0-3, 4-5, 6-7): each group's
    r-burst/rT/p/G/out-DMA runs right after its last wave, so only the
    last 2 batches' output (0.26MB) drains at the end.
  - softmax exp uses accum_out to fuse the row-sum.
  - psum: z x4 (2KB slots, also used by boundary transposes), s x2,
    aux x2 = exactly 8 banks.
"""

import os
import sys

sys.path.insert(0, "/opt/trn_rl_repo")
sys.path.insert(0, "/opt/trn_rl_repo/concourse")

import numpy as np
import ml_dtypes

import concourse.bass as bass
import concourse.mybir as mybir
from concourse import tile
from concourse.bass_utils import run_bass_kernel_spmd

F32 = mybir.dt.float32
BF16 = mybir.dt.bfloat16
BF16_NP = ml_dtypes.bfloat16
TANH = mybir.ActivationFunctionType.Tanh
EXP = mybir.ActivationFunctionType.Exp
FP8 = mybir.dt.float8e4
FP8_NP = ml_dtypes.float8_e4m3
DR = mybir.MatmulPerfMode.DoubleRow
WSCALE = 16.0     # W_h pre-scale into fp8 range
USCALE = 128.0    # u pre-scale into fp8 range
ASCALE = 128.0    # alpha pre-scale (max alpha=1 -> 128 < 240 fp8 max)
RS = 32.0         # r pre-scale into fp8 range
ESC = 8.0         # exp pre-scale into fp8 range (e in (0, ~13))
WPS = 16.0        # W_p pre-scale into fp8 range

B, T, H = 64, 512, 1024
NCORES = 8
PB = B // NCORES          # batches per core = 8
KT = H // 128             # 8 k-tiles over h_in
MT = H // 128             # 8 m-tiles over h_out
KT2 = H // 256            # 4 double-row k-tiles
TT2 = T // 256            # 2 double-row t-tiles for r
GROUPS = [(0, 4), (4, 2), (6, 2)]   # (first batch, size) output groups

_CACHE: dict = {}


def _build_nc() -> bass.Bass:
    nc = bass.Bass()

    xQ8 = nc.declare_dram_parameter("xQ8", [PB, 128, KT2 * 2 * T], FP8, isOutput=False)
    whQ8 = nc.declare_dram_parameter(
        "whQ8", [MT, 128, KT2 * 2 * 128], FP8, isOutput=False
    )
    ht8 = nc.declare_dram_parameter("ht8", [PB, 128, TT2 * 2 * H], FP8, isOutput=False)
    blob = nc.declare_dram_parameter("blob", [128, 4808], mybir.dt.uint8,
                                     isOutput=False)
    wpT8 = nc.declare_dram_parameter("wpT8", [128, KT2 * 2 * H], FP8, isOutput=False)
    wxh = nc.declare_dram_parameter("wxT_hi", [H, H], BF16, isOutput=False)
    hl2 = nc.declare_dram_parameter("hl2", [H, 2 * B], BF16, isOutput=False)
    out = nc.declare_dram_parameter("out", [PB, B, H], BF16, isOutput=True)

    with tile.TileContext(nc) as tc:
        with (
            tc.tile_pool(name="const", bufs=1) as cp,
            tc.tile_pool(name="xchunk", bufs=1) as xp,
            tc.tile_pool(name="hts", bufs=1) as hp,
            tc.tile_pool(name="tz", bufs=6) as tzp,
            tc.tile_pool(name="small", bufs=1) as sp,
            tc.tile_pool(name="sc", bufs=2) as scp,
            tc.tile_pool(name="outp", bufs=4) as op_,
            tc.tile_pool(name="zps", bufs=4, space=bass.MemorySpace.PSUM) as zp,
            tc.tile_pool(name="sps", bufs=2, space=bass.MemorySpace.PSUM) as spp,
            tc.tile_pool(name="aux", bufs=2, space=bass.MemorySpace.PSUM) as ap,
        ):
            # ---- input DMAs: few, large; sync queue in consumption order ----
            blob_sb = cp.tile([128, 4808], mybir.dt.uint8)
            nc.sync.dma_start(blob_sb[:], blob[:])
            bh_sb = blob_sb[:, 0:32].bitcast(F32)
            u8_sb = blob_sb[:, 32:64].bitcast(FP8).rearrange(
                "p (j q) -> p j q", j=2
            )
            selg_sb = blob_sb[:4, 64:576].bitcast(BF16).rearrange(
                "g (q m) -> g q m", q=2
            )
            id_sb = blob_sb[:4, 576:584].bitcast(BF16)
            bpx_sb = blob_sb[:1, 584:4680].bitcast(BF16)
            ones_sb = blob_sb[:1, 4680:4808].bitcast(BF16)

            wm_sb = []

            def _load_wm(m):
                wm = cp.tile([128, KT2, 2, 128], FP8, name=f"wm{m}")
                nc.sync.dma_start(
                    wm[:], whQ8[m].rearrange("p (kt j o) -> p kt j o", j=2, o=128)
                )
                return wm

            xc_sb = []

            def _load_xc(b, split=False):
                xc = xp.tile([128, KT2, 2, T], FP8, name=f"xc{b}")
                halves = ((0, 2), (2, 4)) if split else ((0, 4),)
                for lo, hi in halves:
                    nc.sync.dma_start(
                        xc[:, lo:hi],
                        xQ8[b].rearrange("p (kt j n) -> p kt j n", j=2, n=T)[
                            :, lo:hi
                        ],
                    )
                return xc

            ht_sb = [None] * PB

            wm_sb.append(_load_wm(0))
            xc0 = xp.tile([128, KT2, 2, T], FP8, name="xc0")
            xc1 = xp.tile([128, KT2, 2, T], FP8, name="xc1")
            for lo, hi in ((0, 2), (2, 4)):
                for b, xc in ((0, xc0), (1, xc1)):
                    nc.sync.dma_start(
                        xc[:, lo:hi],
                        xQ8[b].rearrange("p (kt j n) -> p kt j n", j=2, n=T)[
                            :, lo:hi
                        ],
                    )
            xc_sb.append(xc0)
            xc_sb.append(xc1)
            wmrest = cp.tile([128, MT - 1, KT2, 2, 128], FP8)
            nc.sync.dma_start(
                wmrest[:],
                whQ8[1:].rearrange("m p (kt j o) -> p m kt j o", j=2, o=128),
            )
            for m in range(1, MT):
                wm_sb.append(wmrest[:, m - 1])
            for b in (2, 3):
                xc_sb.append(_load_xc(b))
            wxh_sb = cp.tile([128, KT, H], BF16)
            nc.sync.dma_start(wxh_sb[:], wxh[:].rearrange("(kt p) n -> p kt n", p=128))
            hl_sb = cp.tile([128, KT, 2 * B], BF16)
            nc.sync.dma_start(hl_sb[:], hl2[:].rearrange("(kt p) j -> p kt j", p=128))
            hlh_sb = hl_sb[:, :, :B]
            hll_sb = hl_sb[:, :, B:]
            for b in (4, 5):
                xc_sb.append(_load_xc(b))
            htpair = []

            def _load_htpair(hp2):
                htp = hp.tile([128, 2, TT2, 2, H], FP8, name=f"htp{hp2}")
                nc.sync.dma_start(
                    htp[:],
                    ht8[2 * hp2 : 2 * hp2 + 2].rearrange(
                        "b p (tt j h) -> p b tt j h", j=2, h=H
                    ),
                )
                htpair.append(htp)
                ht_sb[2 * hp2] = htp[:, 0]
                ht_sb[2 * hp2 + 1] = htp[:, 1]

            _load_htpair(0)
            _load_htpair(1)
            for b in (6, 7):
                xc_sb.append(_load_xc(b))
            _load_htpair(2)
            _load_htpair(3)
            wpT_sb = cp.tile([128, KT2, 2, H], FP8)
            nc.sync.dma_start(
                wpT_sb[:], wpT8[:].rearrange("p (kt j n) -> p kt j n", j=2, n=H)
            )

            am8 = [None] * len(GROUPS)
            esum_g = [None] * len(GROUPS)
            einv_g = [None] * len(GROUPS)
            x2_sb = sp.tile([128, H], F32)
            s_ps = [None, None]

            def softmax_alpha(b, g, bl):
                """exp(scores) -> e_b + per-group esum, then transpose RAW e
                values into am8 (dep only on the EXP, so the wave-end stall
                is just the EXP latency); 1/esum is applied later as a
                per-row scale on rflat."""
                e_b = scp.tile([1, T], BF16, tag="eb")
                nc.scalar.activation(
                    e_b[:1], s_ps[b % 2][:1], EXP, scale=1.0 / USCALE,
                    accum_out=esum_g[g][:1, bl : bl + 1],
                )
                # t = (tt2*2 + jj)*128 + p: chunk c -> column bl of block bl
                for c in range(4):
                    tp_ps = zp.tile([128, 1], BF16, tag="z", name="tp")
                    nc.tensor.transpose(
                        tp_ps[:, :1], e_b[:1, c * 128 : (c + 1) * 128], id_sb[:1, :1]
                    )
                    nc.vector.tensor_scalar_mul(
                        am8[g][:, bl, c // 2, c % 2, bl : bl + 1], tp_ps[:, :1], ESC
                    )

            def emit_x2(hc):
                """x2 = hlast @ W_x.T + (b_p + b_x), bf16 hi@hi + lo@hi."""
                if True:
                    x_ps = ap.tile([B, 512], F32, tag="aux", name=f"x{hc}")
                    n = 0
                    nmm = 2 * KT + 2
                    for lh in (hlh_sb, hll_sb):
                        for kt in range(KT):
                            nc.tensor.matmul(
                                x_ps[:],
                                lh[:, kt, :],
                                wxh_sb[:, kt, hc * 512 : (hc + 1) * 512],
                                start=(n == 0),
                                stop=(n == nmm - 1),
                            )
                            n += 1
                    for row in range(2):
                        nc.tensor.matmul(
                            x_ps[:],
                            ones_sb[:1, :],
                            bpx_sb[:1, row * H + hc * 512 : row * H + (hc + 1) * 512],
                            start=(n == 0),
                            stop=(n == nmm - 1),
                        )
                        n += 1
                    nc.vector.tensor_scalar_mul(
                        x2_sb[:B, hc * 512 : (hc + 1) * 512], x_ps[:], 1.0
                    )
                    nc.vector.tensor_scalar_mul(
                        x2_sb[B:, hc * 512 : (hc + 1) * 512], x_ps[:], 1.0
                    )

            def boundary_steps(g):
                """r-burst/rT/p/G for group g as steps interleavable into
                the next wave's m-loop."""
                start, size = GROUPS[g]
                st = {}

                def emit_r(hc):
                    if hc == 0:
                        # esum [1,size] -> bf16 -> [size,1] -> einv for rflat
                        es16 = scp.tile([1, 4], BF16, tag="es16")
                        nc.vector.tensor_scalar_mul(
                            es16[:1, :size], esum_g[g][:1, :size], 1.0
                        )
                        et_ps = zp.tile([4, 1], BF16, tag="z", name="et")
                        nc.tensor.transpose(
                            et_ps[:size, :1], es16[:1, :size], id_sb[:1, :1]
                        )
                        einv_g[g] = sp.tile(
                            [4, 1], F32, tag=f"einv{g}", name=f"einv{g}"
                        )
                        nc.vector.reciprocal(einv_g[g][:size], et_ps[:size, :1])
                    r_ps = ap.tile([4, 512], F32, tag="aux", name=f"r{g}{hc}")
                    n = 0
                    for bl in range(size):
                        for tt in range(TT2):
                            nc.tensor.matmul(
                                r_ps[:size],
                                am8[g][:, bl, tt, :, :size],
                                ht_sb[start + bl][:, tt, :, hc * 512 : (hc + 1) * 512],
                                start=(n == 0),
                                stop=(n == 2 * size - 1),
                                perf_mode=DR,
                            )
                            n += 1
                    rflat = sp.tile([4, 512], BF16, tag="rf", name="rflat", bufs=4)
                    nc.vector.tensor_scalar(
                        rflat[:size],
                        r_ps[:size],
                        einv_g[g][:size, :1],
                        RS / ESC,
                        mybir.AluOpType.mult,
                        mybir.AluOpType.mult,
                    )
                    st[f"rfl{hc}"] = rflat

                def emit_rT():
                    # rT: h = (kt2*2+j)*128 + p = (hc*4 + ktl)*128 + p
                    rT8 = sp.tile(
                        [128, KT2, 2, 16], FP8, tag="rT", name=f"rT{g}", bufs=2
                    )
                    tps = []
                    for hc in range(2):
                        for ktl in range(4):
                            tp_ps = zp.tile([128, 4], BF16, tag="z", name="tpr")
                            nc.tensor.transpose(
                                tp_ps[:, :size],
                                st[f"rfl{hc}"][:size, ktl * 128 : (ktl + 1) * 128],
                                id_sb[:size, :size],
                            )
                            tps.append((hc * 4 + ktl, tp_ps))
                    for kk, tp_ps in tps:
                        nc.vector.tensor_scalar_mul(
                            rT8[:, kk // 2, kk % 2, :size], tp_ps[:, :size], 1.0
                        )
                    st["rT8"] = rT8

                def emit_p():
                    p_sb = sp.tile([4, H], BF16, tag="psb", name=f"p{g}", bufs=2)
                    for hc in range(2):
                        p_ps = ap.tile([4, 512], F32, tag="aux", name=f"pp{g}{hc}")
                        for kt in range(KT2):
                            nc.tensor.matmul(
                                p_ps[:size],
                                st["rT8"][:, kt, :, :size],
                                wpT_sb[:, kt, :, hc * 512 : (hc + 1) * 512],
                                start=(kt == 0),
                                stop=(kt == KT2 - 1),
                                perf_mode=DR,
                            )
                        nc.vector.tensor_scalar_mul(
                            p_sb[:size, hc * 512 : (hc + 1) * 512],
                            p_ps[:size],
                            1.0 / (RS * WPS),
                        )
                    st["p_sb"] = p_sb

                def emit_G(q):
                    for hc in range(2):
                        o_ps = ap.tile([128, 512], F32, tag="aux", name=f"o{g}{q}{hc}")
                        nc.tensor.matmul(
                            o_ps[:],
                            selg_sb[:size, q, :],
                            st["p_sb"][:size, hc * 512 : (hc + 1) * 512],
                            start=True,
                            stop=True,
                        )
                        o_sb = op_.tile([128, 512], F32, tag="oadd")
                        nc.vector.tensor_tensor(
                            o_sb[:],
                            o_ps[:],
                            x2_sb[:, hc * 512 : (hc + 1) * 512],
                            mybir.AluOpType.add,
                        )
                        o2 = op_.tile([128, 512], BF16, tag="otanh")
                        nc.scalar.activation(o2[:], o_sb[:], TANH)
                        i0 = start + 2 * q
                        nc.scalar.dma_start(
                            out[i0 : i0 + 2, :, hc * 512 : (hc + 1) * 512].rearrange(
                                "i j h -> (i j) h"
                            ),
                            o2[:],
                        )

                steps = [lambda: emit_r(0), lambda: emit_r(1), emit_rT, emit_p]
                for q in range(size // 2):
                    steps.append(lambda q=q: emit_G(q))
                return steps

            # ---- main loop: waves of 2 batches; boundary/x2 steps are
            #      interleaved one per m-iteration to keep the PE dense ----
            pending = []
            for w in range(PB // 2):
                b0 = 2 * w
                g = next(i for i, (s, n) in enumerate(GROUPS) if s <= b0 < s + n)
                gstart, gsize = GROUPS[g]
                if b0 == gstart:
                    am = sp.tile(
                        [128, 4, TT2, 2, 16], FP8, tag=f"am{g}", name=f"am8_{g}"
                    )  # [p, bl, tt2, j, col16]: j-stride 16 for DR ldweights
                    nc.vector.memset(am[:], 0.0)
                    am8[g] = am
                    esum_g[g] = sp.tile([1, 4], F32, tag=f"esum{g}", name=f"es{g}")
                s_ps[0] = spp.tile([1, T], F32, tag="s", name=f"s{b0}")
                s_ps[1] = spp.tile([1, T], F32, tag="s", name=f"s{b0 + 1}")
                tz8 = [None, None]
                for m in range(MT):
                    if pending and m >= 2:
                        pending.pop(0)()
                    z_ps = [
                        zp.tile([128, T], F32, tag="z", name=f"z{b0 + b2}_{m}")
                        for b2 in range(2)
                    ]
                    for kt in range(KT2):
                        for b2 in range(2):
                            nc.tensor.matmul(
                                z_ps[b2][:],
                                wm_sb[m][:, kt, :, :],
                                xc_sb[b0 + b2][:, kt, :, :],
                                start=(kt == 0),
                                stop=(kt == KT2 - 1),
                                perf_mode=DR,
                            )
                    q, jj = m // 2, m % 2
                    for b2 in range(2):
                        if jj == 0:
                            tz8[b2] = tzp.tile(
                                [128, 2, T], FP8, tag="tz8", name=f"tz{b0+b2}_{q}"
                            )
                        nc.scalar.activation(
                            tz8[b2][:, jj, :],
                            z_ps[b2][:],
                            TANH,
                            bias=bh_sb[:, m : m + 1],
                            scale=1.0 / WSCALE,
                        )
                    if jj == 1:
                        for b2 in range(2):
                            nc.tensor.matmul(
                                s_ps[b2][:1, :],
                                u8_sb[:, :, q : q + 1],
                                tz8[b2][:],
                                start=(q == 0),
                                stop=(q == KT2 - 1),
                                perf_mode=DR,
                            )
                for b2 in range(2):
                    b = b0 + b2
                    softmax_alpha(b, g, b - gstart)
                for step in pending:
                    step()
                pending = []
                if w == 0:
                    pending = [lambda: emit_x2(0), lambda: emit_x2(1)]
                if b0 + 2 == gstart + gsize:
                    pending.extend(boundary_steps(g))
            for step in pending:
                step()
    _split_excess_waits(nc)
    return nc


def _split_excess_waits(nc: bass.Bass, max_waits: int = 1) -> None:
    """Walrus's per-instruction sync-wait slots are limited; move excess
    on_wait entries onto wait-only NoOps inserted just before the
    instruction (same engine, so ordering is preserved)."""
    for fn in nc.m.functions:
        for blk in fn.blocks:
            new = []
            for inst in blk.instructions:
                si = inst.sync_info
                waits = list(si.on_wait) if si is not None and si.on_wait else []
                if len(waits) > max_waits:
                    extra, keep = waits[:-max_waits], waits[-max_waits:]
                    for ci in range(0, len(extra), max_waits):
                        nop = mybir.InstNoOp(
                            name=f"{inst.name}-wsplit{ci}", ins=[], outs=[]
                        )
                        nop.engine = inst.engine
                        nop.sync_info = mybir.SyncInfo(
                            on_wait=extra[ci : ci + max_waits], on_update=[]
                        )
                        new.append(nop)
                    inst.sync_info = mybir.SyncInfo(
                        on_wait=keep, on_update=list(si.on_update or [])
                    )
                new.append(inst)
            blk.instructions[:] = new


def _split_bf16(a: np.ndarray) -> tuple[np.ndarray, np.ndarray]:
    hi = a.astype(BF16_NP)
    lo = (a - hi.astype(np.float32)).astype(BF16_NP)
    return hi, lo


def _pad_u8(w_w: np.ndarray) -> np.ndarray:
    u = np.zeros((128, 2, 16), np.float32)
    u[:, :, :KT2] = (w_w[0, :H] * USCALE).reshape(KT2, 2, 128).transpose(2, 1, 0)
    return np.ascontiguousarray(u.reshape(128, 2 * 16)).astype(FP8_NP)


def _host_prep(inputs: dict) -> list[dict]:
    hidden = np.asarray(inputs["hidden"], np.float32)
    W_h = np.asarray(inputs["W_h"], np.float32)
    b_h = np.asarray(inputs["b_h"], np.float32)
    w_w = np.asarray(inputs["w_w"], np.float32)
    W_p = np.asarray(inputs["W_p"], np.float32)
    b_p = np.asarray(inputs["b_p"], np.float32)
    W_x = np.asarray(inputs["W_x"], np.float32)
    b_x = np.asarray(inputs["b_x"], np.float32)

    # G selector: row i = 2q + m//64 (q=0 block alone serves 2-row groups)
    selgm = np.zeros((4, 2, 128), np.float32)
    for q in range(2):
        for m in range(128):
            selgm[2 * q + m // 64, q, m] = 1.0

    wxT = np.ascontiguousarray(W_x.T)
    wx_hi, _ = _split_bf16(wxT)
    hlT = np.ascontiguousarray(hidden[:, -1, :].T)
    hl_hi, hl_lo = _split_bf16(hlT)
    bpx_hi, bpx_lo = _split_bf16((b_p + b_x).reshape(1, H))

    bhm = np.ascontiguousarray(b_h.reshape(MT, 128).T, np.float32)
    blob = np.zeros((128, 4808), np.uint8)
    blob[:, 0:32] = bhm.view(np.uint8).reshape(128, 32)
    blob[:, 32:64] = _pad_u8(w_w).view(np.uint8).reshape(128, 32)
    selg_b = selgm.reshape(4, 2 * 128).astype(BF16_NP)
    blob[:4, 64:576] = selg_b.view(np.uint8).reshape(4, 512)
    ident_b = np.eye(4, dtype=np.float32).astype(BF16_NP)
    blob[:4, 576:584] = ident_b.view(np.uint8).reshape(4, 8)
    bpx_b = np.concatenate([bpx_hi, bpx_lo], axis=1)
    blob[0, 584:4680] = bpx_b.view(np.uint8).reshape(4096)
    blob[0, 4680:4808] = np.ones((1, B), BF16_NP).view(np.uint8).reshape(128)
    shared = {
        "whQ8": np.ascontiguousarray(
            (W_h.T * WSCALE)
            .reshape(KT2, 128, 2, MT, 128)
            .transpose(3, 1, 0, 2, 4)
            .reshape(MT, 128, KT2 * 2 * 128)
        ).astype(FP8_NP),
        "blob": blob,
        # wpT8[p, kt2, j, n] = W_p.T[(kt2*2+j)*128 + p, n] * WPS
        "wpT8": np.ascontiguousarray(
            (W_p.T * WPS).reshape(KT2, 2, 128, H).transpose(2, 0, 1, 3).reshape(
                128, KT2 * 2 * H
            )
        ).astype(FP8_NP),
        "wxT_hi": wx_hi,
        "hl2": np.ascontiguousarray(np.concatenate([hl_hi, hl_lo], axis=1)),
    }

    in_maps = []
    for c in range(NCORES):
        slab = hidden[c * PB : (c + 1) * PB]          # [PB, T, H]
        m = dict(shared)
        m["xQ8"] = np.ascontiguousarray(
            slab.reshape(PB, T, KT2, 128, 2)
            .transpose(0, 3, 2, 4, 1)
            .reshape(PB, 128, KT2 * 2 * T)
        ).astype(FP8_NP)
        # ht8[b][p, tt2, j, h] = hidden[b, t = (tt2*2+j)*128 + p, h]
        m["ht8"] = np.ascontiguousarray(
            slab.reshape(PB, TT2, 2, 128, H)
            .transpose(0, 3, 1, 2, 4)
            .reshape(PB, 128, TT2 * 2 * H)
        ).astype(FP8_NP)
        in_maps.append(m)
    return in_maps


def _ensure_ntff_hook() -> None:
    """The agent image's antenv lacks axon_hooks; register a shim module
    wired to the libaxon NTFF profile hook so trace=True works."""
    try:
        from antenv.axon_hooks import get_axon_ntff_profile_hook  # noqa: F401
        return
    except ImportError:
        pass
    import types
    import antenv
    from trn_agent_boot.trn_boot import _ntff_profile_via_ctypes

    mod = types.ModuleType("antenv.axon_hooks")
    holder = {"hook": _ntff_profile_via_ctypes("/opt/axon/libaxon_pjrt.so")}
    mod.get_axon_ntff_profile_hook = lambda: holder["hook"]
    mod.set_axon_ntff_profile_hook = lambda h: holder.__setitem__("hook", h)
    sys.modules["antenv.axon_hooks"] = mod
    antenv.axon_hooks = mod


def run(inputs: dict, trace: bool = False, **kw):
    if trace:
        _ensure_ntff_hook()
    if "nc" not in _CACHE:
        _CACHE["nc"] = _build_nc()
    nc = _CACHE["nc"]
    in_maps = _host_prep(inputs)
    res = run_bass_kernel_spmd(nc, in_maps, list(range(NCORES)), trace=trace, **kw)
    out = np.empty((B, B, H), np.float32)
    for c in range(NCORES):
        out[c * PB : (c + 1) * PB] = np.asarray(res.results[c]["out"], np.float32)
    return out, res


def kernel(**inputs) -> np.ndarray:
    out, _ = run(inputs)
    return out


# revision 20
# speedup vs baseline: 1.2356x; 1.0151x over previous
"""TRN2 Bass kernel for nn_Attention_76802605187492.

Math (B=64, T=512, H=1024, A=300):
  The aspect branch only adds a per-batch constant to the attention
  scores, which softmax cancels, so it does not affect the output.
  Per batch b:
    scores[t] = u . tanh(W_h hidden[b,t] + b_h)      u = w_w[0, :H]
    alpha     = softmax_t(scores)
    r         = sum_t alpha[t] hidden[b,t]
    p_b       = r @ W_p.T
    x_j       = hidden[j,-1] @ W_x.T                  (all j)
    out[b,j]  = tanh(p_b + x_j + (b_p + b_x))         -> [B, B, H]

Sharding: data-parallel over batch across 8 cores (8 batches each).

v4 design (vs 139us baseline):
  - big matmul, scores, r and p all in fp8 DoubleRow (2 k-rows/cycle).
    Scores pair m-tiles on the j dim (tanh writes fp8 tz directly);
    r contracts t via an fp8 t-layout upload ht8 (t = (tt2*2+j)*128+p),
    alpha transposed into masked am8 columns, 4 chunks per batch.
    DR ldweights requires the j-plane stride to be >= 16 elements
    (u8/rT8/am8 padded accordingly).
  - batches processed in waves of 2. The SP engine issues DMA
    descriptors at ~1us each and the Tile scheduler hoists dep-free
    issues, so ALL inputs ride the single sync queue as ~19 large DMAs
    in consumption-priority order (the queue executes descriptors
    FIFO); outputs go on the scalar queue.
  - boundary work (r-burst/rT/p/G) is interleaved one step per
    m-iteration into the NEXT wave's matmul stream so the PE never
    idles long enough for the HAM clock-gate to throttle it.
  - x2 (hlast @ W_x.T, bf16 hi@hi + lo@hi; bias rides via k=1 ones
    matmuls) is issued right after wave 0 where the PE is DMA-starved.
  - output pipelined in 3 groups (batches 0-3, 4-5, 6-7): each group's
    r-burst/rT/p/G/out-DMA runs right after its last wave, so only the
    last 2 batches' output (0.26MB) drains at the end.
  - softmax exp uses accum_out to fuse the row-sum.
  - psum: z x4 (2KB slots, also used by boundary transposes), s x2,
    aux x2 = exactly 8 banks.
"""

import os
import sys

sys.path.insert(0, "/opt/trn_rl_repo")
sys.path.insert(0, "/opt/trn_rl_repo/concourse")

import numpy as np
import ml_dtypes

import concourse.bass as bass
import concourse.mybir as mybir
from concourse import tile
from concourse.bass_utils import run_bass_kernel_spmd

F32 = mybir.dt.float32
BF16 = mybir.dt.bfloat16
BF16_NP = ml_dtypes.bfloat16
TANH = mybir.ActivationFunctionType.Tanh
EXP = mybir.ActivationFunctionType.Exp
FP8 = mybir.dt.float8e4
FP8_NP = ml_dtypes.float8_e4m3
DR = mybir.MatmulPerfMode.DoubleRow
WSCALE = 16.0     # W_h pre-scale into fp8 range
USCALE = 128.0    # u pre-scale into fp8 range
ASCALE = 128.0    # alpha pre-scale (max alpha=1 -> 128 < 240 fp8 max)
RS = 32.0         # r pre-scale into fp8 range
ESC = 8.0         # exp pre-scale into fp8 range (e in (0, ~13))
WPS = 16.0        # W_p pre-scale into fp8 range

B, T, H = 64, 512, 1024
NCORES = 8
PB = B // NCORES          # batches per core = 8
KT = H // 128             # 8 k-tiles over h_in
MT = H // 128             # 8 m-tiles over h_out
KT2 = H // 256            # 4 double-row k-tiles
TT2 = T // 256            # 2 double-row t-tiles for r
GROUPS = [(0, 4), (4, 2), (6, 2)]   # (first batch, size) output groups

_CACHE: dict = {}


def _build_nc() -> bass.Bass:
    nc = bass.Bass()

    xQ8 = nc.declare_dram_parameter("xQ8", [PB, 128, KT2 * 2 * T], FP8, isOutput=False)
    whQ8 = nc.declare_dram_parameter(
        "whQ8", [MT, 128, KT2 * 2 * 128], FP8, isOutput=False
    )
    ht8 = nc.declare_dram_parameter("ht8", [PB, 128, TT2 * 2 * H], FP8, isOutput=False)
    blobA = nc.declare_dram_parameter("blobA", [128, 72], mybir.dt.uint8,
                                      isOutput=False)
    blobB = nc.declare_dram_parameter("blobB", [4, 4736], mybir.dt.uint8,
                                      isOutput=False)
    wpT8 = nc.declare_dram_parameter("wpT8", [128, KT2 * 2 * H], FP8, isOutput=False)
    wxh = nc.declare_dram_parameter("wxT_hi", [H, H], BF16, isOutput=False)
    hl2 = nc.declare_dram_parameter("hl2", [H, 2 * B], BF16, isOutput=False)
    out = nc.declare_dram_parameter("out", [PB, B, H], BF16, isOutput=True)

    with tile.TileContext(nc) as tc:
        with (
            tc.tile_pool(name="const", bufs=1) as cp,
            tc.tile_pool(name="xchunk", bufs=1) as xp,
            tc.tile_pool(name="hts", bufs=1) as hp,
            tc.tile_pool(name="tz", bufs=6) as tzp,
            tc.tile_pool(name="small", bufs=1) as sp,
            tc.tile_pool(name="sc", bufs=2) as scp,
            tc.tile_pool(name="outp", bufs=4) as op_,
            tc.tile_pool(name="zps", bufs=4, space=bass.MemorySpace.PSUM) as zp,
            tc.tile_pool(name="sps", bufs=2, space=bass.MemorySpace.PSUM) as spp,
            tc.tile_pool(name="aux", bufs=2, space=bass.MemorySpace.PSUM) as ap,
        ):
            # ---- input DMAs: few, large; sync queue in consumption order.
            #      blobA (9KB) carries only what the first waves touch; the
            #      wide selg/bpx/ones blob is deferred past the xc stream ----
            blobA_sb = cp.tile([128, 72], mybir.dt.uint8)
            nc.sync.dma_start(blobA_sb[:], blobA[:])
            bh_sb = blobA_sb[:, 0:32].bitcast(F32)
            u8_sb = blobA_sb[:, 32:64].bitcast(FP8).rearrange(
                "p (j q) -> p j q", j=2
            )
            id_sb = blobA_sb[:4, 64:72].bitcast(BF16)

            wm_sb = []

            def _load_wm(m):
                wm = cp.tile([128, KT2, 2, 128], FP8, name=f"wm{m}")
                nc.sync.dma_start(
                    wm[:], whQ8[m].rearrange("p (kt j o) -> p kt j o", j=2, o=128)
                )
                return wm

            xc_sb = []

            def _load_xc(b, split=False):
                xc = xp.tile([128, KT2, 2, T], FP8, name=f"xc{b}")
                halves = ((0, 2), (2, 4)) if split else ((0, 4),)
                for lo, hi in halves:
                    nc.sync.dma_start(
                        xc[:, lo:hi],
                        xQ8[b].rearrange("p (kt j n) -> p kt j n", j=2, n=T)[
                            :, lo:hi
                        ],
                    )
                return xc

            ht_sb = [None] * PB

            wm_sb.append(_load_wm(0))
            xc0 = xp.tile([128, KT2, 2, T], FP8, name="xc0")
            xc1 = xp.tile([128, KT2, 2, T], FP8, name="xc1")
            for lo, hi in ((0, 2), (2, 4)):
                for b, xc in ((0, xc0), (1, xc1)):
                    nc.sync.dma_start(
                        xc[:, lo:hi],
                        xQ8[b].rearrange("p (kt j n) -> p kt j n", j=2, n=T)[
                            :, lo:hi
                        ],
                    )
            xc_sb.append(xc0)
            xc_sb.append(xc1)
            wmA = cp.tile([128, 3, KT2, 2, 128], FP8)
            nc.sync.dma_start(
                wmA[:],
                whQ8[1:4].rearrange("m p (kt j o) -> p m kt j o", j=2, o=128),
            )
            wmB = cp.tile([128, 4, KT2, 2, 128], FP8)
            nc.sync.dma_start(
                wmB[:],
                whQ8[4:].rearrange("m p (kt j o) -> p m kt j o", j=2, o=128),
            )
            for m in range(1, 4):
                wm_sb.append(wmA[:, m - 1])
            for m in range(4, MT):
                wm_sb.append(wmB[:, m - 4])
            for b in (2, 3):
                xc_sb.append(_load_xc(b))
            blobB_sb = cp.tile([4, 4736], mybir.dt.uint8)
            nc.sync.dma_start(blobB_sb[:], blobB[:])
            selg_sb = blobB_sb[:4, 0:512].bitcast(BF16).rearrange(
                "g (q m) -> g q m", q=2
            )
            bpx_sb = blobB_sb[:1, 512:4608].bitcast(BF16)
            ones_sb = blobB_sb[:1, 4608:4736].bitcast(BF16)
            wxh_sb = cp.tile([128, KT, H], BF16)
            nc.sync.dma_start(wxh_sb[:], wxh[:].rearrange("(kt p) n -> p kt n", p=128))
            hl_sb = cp.tile([128, KT, 2 * B], BF16)
            nc.sync.dma_start(hl_sb[:], hl2[:].rearrange("(kt p) j -> p kt j", p=128))
            hlh_sb = hl_sb[:, :, :B]
            hll_sb = hl_sb[:, :, B:]
            for b in (4, 5):
                xc_sb.append(_load_xc(b))
            htpair = []

            def _load_htpair(hp2):
                htp = hp.tile([128, 2, TT2, 2, H], FP8, name=f"htp{hp2}")
                nc.sync.dma_start(
                    htp[:],
                    ht8[2 * hp2 : 2 * hp2 + 2].rearrange(
                        "b p (tt j h) -> p b tt j h", j=2, h=H
                    ),
                )
                htpair.append(htp)
                ht_sb[2 * hp2] = htp[:, 0]
                ht_sb[2 * hp2 + 1] = htp[:, 1]

            _load_htpair(0)
            _load_htpair(1)
            for b in (6, 7):
                xc_sb.append(_load_xc(b))
            _load_htpair(2)
            _load_htpair(3)
            wpT_sb = cp.tile([128, KT2, 2, H], FP8)
            nc.sync.dma_start(
                wpT_sb[:], wpT8[:].rearrange("p (kt j n) -> p kt j n", j=2, n=H)
            )

            am8 = [None] * len(GROUPS)
            esum_g = [None] * len(GROUPS)
            einv_g = [None] * len(GROUPS)
            x2_sb = sp.tile([128, H], F32)
            s_ps = [None, None]

            def softmax_alpha(b, g, bl):
                """exp(scores) -> e_b + per-group esum, then transpose RAW e
                values into am8 (dep only on the EXP, so the wave-end stall
                is just the EXP latency); 1/esum is applied later as a
                per-row scale on rflat."""
                e_b = scp.tile([1, T], BF16, tag="eb")
                nc.scalar.activation(
                    e_b[:1], s_ps[b % 2][:1], EXP, scale=1.0 / USCALE,
                    accum_out=esum_g[g][:1, bl : bl + 1],
                )
                # t = (tt2*2 + jj)*128 + p: chunk c -> column bl of block bl
                for c in range(4):
                    tp_ps = zp.tile([128, 1], BF16, tag="z", name="tp")
                    nc.tensor.transpose(
                        tp_ps[:, :1], e_b[:1, c * 128 : (c + 1) * 128], id_sb[:1, :1]
                    )
                    nc.vector.tensor_scalar_mul(
                        am8[g][:, bl, c // 2, c % 2, bl : bl + 1], tp_ps[:, :1], ESC
                    )

            def emit_x2(hc):
                """x2 = hlast @ W_x.T + (b_p + b_x), bf16 hi@hi + lo@hi."""
                if True:
                    x_ps = ap.tile([B, 512], F32, tag="aux", name=f"x{hc}")
                    n = 0
                    nmm = 2 * KT + 2
                    for lh in (hlh_sb, hll_sb):
                        for kt in range(KT):
                            nc.tensor.matmul(
                                x_ps[:],
                                lh[:, kt, :],
                                wxh_sb[:, kt, hc * 512 : (hc + 1) * 512],
                                start=(n == 0),
                                stop=(n == nmm - 1),
                            )
                            n += 1
                    for row in range(2):
                        nc.tensor.matmul(
                            x_ps[:],
                            ones_sb[:1, :],
                            bpx_sb[:1, row * H + hc * 512 : row * H + (hc + 1) * 512],
                            start=(n == 0),
                            stop=(n == nmm - 1),
                        )
                        n += 1
                    nc.vector.tensor_scalar_mul(
                        x2_sb[:B, hc * 512 : (hc + 1) * 512], x_ps[:], 1.0
                    )
                    nc.vector.tensor_scalar_mul(
                        x2_sb[B:, hc * 512 : (hc + 1) * 512], x_ps[:], 1.0
                    )

            def boundary_steps(g):
                """r-burst/rT/p/G for group g as steps interleavable into
                the next wave's m-loop."""
                start, size = GROUPS[g]
                st = {}

                def emit_r(hc):
                    if hc == 0:
                        # esum [1,size] -> bf16 -> [size,1] -> einv for rflat
                        es16 = scp.tile([1, 4], BF16, tag="es16")
                        nc.vector.tensor_scalar_mul(
                            es16[:1, :size], esum_g[g][:1, :size], 1.0
                        )
                        et_ps = zp.tile([4, 1], BF16, tag="z", name="et")
                        nc.tensor.transpose(
                            et_ps[:size, :1], es16[:1, :size], id_sb[:1, :1]
                        )
                        einv_g[g] = sp.tile(
                            [4, 1], F32, tag=f"einv{g}", name=f"einv{g}"
                        )
                        nc.vector.reciprocal(einv_g[g][:size], et_ps[:size, :1])
                    r_ps = ap.tile([4, 512], F32, tag="aux", name=f"r{g}{hc}")
                    n = 0
                    for bl in range(size):
                        for tt in range(TT2):
                            nc.tensor.matmul(
                                r_ps[:size],
                                am8[g][:, bl, tt, :, :size],
                                ht_sb[start + bl][:, tt, :, hc * 512 : (hc + 1) * 512],
                                start=(n == 0),
                                stop=(n == 2 * size - 1),
                                perf_mode=DR,
                            )
                            n += 1
                    rflat = sp.tile([4, 512], BF16, tag="rf", name="rflat", bufs=4)
                    nc.vector.tensor_scalar(
                        rflat[:size],
                        r_ps[:size],
                        einv_g[g][:size, :1],
                        RS / ESC,
                        mybir.AluOpType.mult,
                        mybir.AluOpType.mult,
                    )
                    st[f"rfl{hc}"] = rflat

                def emit_rT():
                    # rT: h = (kt2*2+j)*128 + p = (hc*4 + ktl)*128 + p
                    rT8 = sp.tile(
                        [128, KT2, 2, 16], FP8, tag="rT", name=f"rT{g}", bufs=2
                    )
                    tps = []
                    for hc in range(2):
                        for ktl in range(4):
                            tp_ps = zp.tile([128, 4], BF16, tag="z", name="tpr")
                            nc.tensor.transpose(
                                tp_ps[:, :size],
                                st[f"rfl{hc}"][:size, ktl * 128 : (ktl + 1) * 128],
                                id_sb[:size, :size],
                            )
                            tps.append((hc * 4 + ktl, tp_ps))
                    for kk, tp_ps in tps:
                        nc.vector.tensor_scalar_mul(
                            rT8[:, kk // 2, kk % 2, :size], tp_ps[:, :size], 1.0
                        )
                    st["rT8"] = rT8

                def emit_p():
                    p_sb = sp.tile([4, H], BF16, tag="psb", name=f"p{g}", bufs=2)
                    for hc in range(2):
                        p_ps = ap.tile([4, 512], F32, tag="aux", name=f"pp{g}{hc}")
                        for kt in range(KT2):
                            nc.tensor.matmul(
                                p_ps[:size],
                                st["rT8"][:, kt, :, :size],
                                wpT_sb[:, kt, :, hc * 512 : (hc + 1) * 512],
                                start=(kt == 0),
                                stop=(kt == KT2 - 1),
                                perf_mode=DR,
                            )
                        nc.vector.tensor_scalar_mul(
                            p_sb[:size, hc * 512 : (hc + 1) * 512],
                            p_ps[:size],
                            1.0 / (RS * WPS),
                        )
                    st["p_sb"] = p_sb

                def emit_G(q):
                    for hc in range(2):
                        o_ps = ap.tile([128, 512], F32, tag="aux", name=f"o{g}{q}{hc}")
                        nc.tensor.matmul(
                            o_ps[:],
                            selg_sb[:size, q, :],
                            st["p_sb"][:size, hc * 512 : (hc + 1) * 512],
                            start=True,
                            stop=True,
                        )
                        o_sb = op_.tile([128, 512], F32, tag="oadd")
                        nc.vector.tensor_tensor(
                            o_sb[:],
                            o_ps[:],
                            x2_sb[:, hc * 512 : (hc + 1) * 512],
                            mybir.AluOpType.add,
                        )
                        o2 = op_.tile([128, 512], BF16, tag="otanh")
                        nc.scalar.activation(o2[:], o_sb[:], TANH)
                        i0 = start + 2 * q
                        nc.scalar.dma_start(
                            out[i0 : i0 + 2, :, hc * 512 : (hc + 1) * 512].rearrange(
                                "i j h -> (i j) h"
                            ),
                            o2[:],
                        )

                steps = [lambda: emit_r(0), lambda: emit_r(1), emit_rT, emit_p]
                for q in range(size // 2):
                    steps.append(lambda q=q: emit_G(q))
                return steps

            # ---- main loop: waves of 2 batches; boundary/x2 steps are
            #      interleaved one per m-iteration to keep the PE dense ----
            pending = []
            for w in range(PB // 2):
                b0 = 2 * w
                g = next(i for i, (s, n) in enumerate(GROUPS) if s <= b0 < s + n)
                gstart, gsize = GROUPS[g]
                if b0 == gstart:
                    am = sp.tile(
                        [128, 4, TT2, 2, 16], FP8, tag=f"am{g}", name=f"am8_{g}"
                    )  # [p, bl, tt2, j, col16]: j-stride 16 for DR ldweights
                    nc.vector.memset(am[:], 0.0)
                    am8[g] = am
                    esum_g[g] = sp.tile([1, 4], F32, tag=f"esum{g}", name=f"es{g}")
                s_ps[0] = spp.tile([1, T], F32, tag="s", name=f"s{b0}")
                s_ps[1] = spp.tile([1, T], F32, tag="s", name=f"s{b0 + 1}")
                tz8 = [None, None]
                for m in range(MT):
                    if pending and m >= 2:
                        pending.pop(0)()
                    z_ps = [
                        zp.tile([128, T], F32, tag="z", name=f"z{b0 + b2}_{m}")
                        for b2 in range(2)
                    ]
                    for kt in range(KT2):
                        for b2 in range(2):
                            nc.tensor.matmul(
                                z_ps[b2][:],
                                wm_sb[m][:, kt, :, :],
                                xc_sb[b0 + b2][:, kt, :, :],
                                start=(kt == 0),
                                stop=(kt == KT2 - 1),
                                perf_mode=DR,
                            )
                    q, jj = m // 2, m % 2
                    for b2 in range(2):
                        if jj == 0:
                            tz8[b2] = tzp.tile(
                                [128, 2, T], FP8, tag="tz8", name=f"tz{b0+b2}_{q}"
                            )
                        nc.scalar.activation(
                            tz8[b2][:, jj, :],
                            z_ps[b2][:],
                            TANH,
                            bias=bh_sb[:, m : m + 1],
                            scale=1.0 / WSCALE,
                        )
                    if jj == 1:
                        for b2 in range(2):
                            nc.tensor.matmul(
                                s_ps[b2][:1, :],
                                u8_sb[:, :, q : q + 1],
                                tz8[b2][:],
                                start=(q == 0),
                                stop=(q == KT2 - 1),
                                perf_mode=DR,
                            )
                for b2 in range(2):
                    b = b0 + b2
                    softmax_alpha(b, g, b - gstart)
                for step in pending:
                    step()
                pending = []
                if w == 0:
                    pending = [lambda: emit_x2(0), lambda: emit_x2(1)]
                if b0 + 2 == gstart + gsize:
                    pending.extend(boundary_steps(g))
            for step in pending:
                step()
    _split_excess_waits(nc)
    return nc


def _split_excess_waits(nc: bass.Bass, max_waits: int = 1) -> None:
    """Walrus's per-instruction sync-wait slots are limited; move excess
    on_wait entries onto wait-only NoOps inserted just before the
    instruction (same engine, so ordering is preserved)."""
    for fn in nc.m.functions:
        for blk in fn.blocks:
            new = []
            for inst in blk.instructions:
                si = inst.sync_info
                waits = list(si.on_wait) if si is not None and si.on_wait else []
                if len(waits) > max_waits:
                    extra, keep = waits[:-max_waits], waits[-max_waits:]
                    for ci in range(0, len(extra), max_waits):
                        nop = mybir.InstNoOp(
                            name=f"{inst.name}-wsplit{ci}", ins=[], outs=[]
                        )
                        nop.engine = inst.engine
                        nop.sync_info = mybir.SyncInfo(
                            on_wait=extra[ci : ci + max_waits], on_update=[]
                        )
                        new.append(nop)
                    inst.sync_info = mybir.SyncInfo(
                        on_wait=keep, on_update=list(si.on_update or [])
                    )
                new.append(inst)
            blk.instructions[:] = new


def _split_bf16(a: np.ndarray) -> tuple[np.ndarray, np.ndarray]:
    hi = a.astype(BF16_NP)
    lo = (a - hi.astype(np.float32)).astype(BF16_NP)
    return hi, lo


def _pad_u8(w_w: np.ndarray) -> np.ndarray:
    u = np.zeros((128, 2, 16), np.float32)
    u[:, :, :KT2] = (w_w[0, :H] * USCALE).reshape(KT2, 2, 128).transpose(2, 1, 0)
    return np.ascontiguousarray(u.reshape(128, 2 * 16)).astype(FP8_NP)


def _host_prep(inputs: dict) -> list[dict]:
    hidden = np.asarray(inputs["hidden"], np.float32)
    W_h = np.asarray(inputs["W_h"], np.float32)
    b_h = np.asarray(inputs["b_h"], np.float32)
    w_w = np.asarray(inputs["w_w"], np.float32)
    W_p = np.asarray(inputs["W_p"], np.float32)
    b_p = np.asarray(inputs["b_p"], np.float32)
    W_x = np.asarray(inputs["W_x"], np.float32)
    b_x = np.asarray(inputs["b_x"], np.float32)

    # G selector: row i = 2q + m//64 (q=0 block alone serves 2-row groups)
    selgm = np.zeros((4, 2, 128), np.float32)
    for q in range(2):
        for m in range(128):
            selgm[2 * q + m // 64, q, m] = 1.0

    wxT = np.ascontiguousarray(W_x.T)
    wx_hi, _ = _split_bf16(wxT)
    hlT = np.ascontiguousarray(hidden[:, -1, :].T)
    hl_hi, hl_lo = _split_bf16(hlT)
    bpx_hi, bpx_lo = _split_bf16((b_p + b_x).reshape(1, H))

    bhm = np.ascontiguousarray(b_h.reshape(MT, 128).T, np.float32)
    blobA = np.zeros((128, 72), np.uint8)
    blobA[:, 0:32] = bhm.view(np.uint8).reshape(128, 32)
    blobA[:, 32:64] = _pad_u8(w_w).view(np.uint8).reshape(128, 32)
    ident_b = np.eye(4, dtype=np.float32).astype(BF16_NP)
    blobA[:4, 64:72] = ident_b.view(np.uint8).reshape(4, 8)
    blobB = np.zeros((4, 4736), np.uint8)
    selg_b = selgm.reshape(4, 2 * 128).astype(BF16_NP)
    blobB[:4, 0:512] = selg_b.view(np.uint8).reshape(4, 512)
    bpx_b = np.concatenate([bpx_hi, bpx_lo], axis=1)
    blobB[0, 512:4608] = bpx_b.view(np.uint8).reshape(4096)
    blobB[0, 4608:4736] = np.ones((1, B), BF16_NP).view(np.uint8).reshape(128)
    shared = {
        "whQ8": np.ascontiguousarray(
            (W_h.T * WSCALE)
            .reshape(KT2, 128, 2, MT, 128)
            .transpose(3, 1, 0, 2, 4)
            .reshape(MT, 128, KT2 * 2 * 128)
        ).astype(FP8_NP),
        "blobA": blobA,
        "blobB": blobB,
        # wpT8[p, kt2, j, n] = W_p.T[(kt2*2+j)*128 + p, n] * WPS
        "wpT8": np.ascontiguousarray(
            (W_p.T * WPS).reshape(KT2, 2, 128, H).transpose(2, 0, 1, 3).reshape(
                128, KT2 * 2 * H
            )
        ).astype(FP8_NP),
        "wxT_hi": wx_hi,
        "hl2": np.ascontiguousarray(np.concatenate([hl_hi, hl_lo], axis=1)),
    }

    in_maps = []
    for c in range(NCORES):
        slab = hidden[c * PB : (c + 1) * PB]          # [PB, T, H]
        m = dict(shared)
        m["xQ8"] = np.ascontiguousarray(
            slab.reshape(PB, T, KT2, 128, 2)
            .transpose(0, 3, 2, 4, 1)
            .reshape(PB, 128, KT2 * 2 * T)
        ).astype(FP8_NP)
        # ht8[b][p, tt2, j, h] = hidden[b, t = (tt2*2+j)*128 + p, h]
        m["ht8"] = np.ascontiguousarray(
            slab.reshape(PB, TT2, 2, 128, H)
            .transpose(0, 3, 1, 2, 4)
            .reshape(PB, 128, TT2 * 2 * H)
        ).astype(FP8_NP)
        in_maps.append(m)
    return in_maps


def _ensure_ntff_hook() -> None:
    """The agent image's antenv lacks axon_hooks; register a shim module
    wired to the libaxon NTFF profile hook so trace=True works."""
    try:
        from antenv.axon_hooks import get_axon_ntff_profile_hook  # noqa: F401
        return
    except ImportError:
        pass
    import types
    import antenv
    from trn_agent_boot.trn_boot import _ntff_profile_via_ctypes

    mod = types.ModuleType("antenv.axon_hooks")
    holder = {"hook": _ntff_profile_via_ctypes("/opt/axon/libaxon_pjrt.so")}
    mod.get_axon_ntff_profile_hook = lambda: holder["hook"]
    mod.set_axon_ntff_profile_hook = lambda h: holder.__setitem__("hook", h)
    sys.modules["antenv.axon_hooks"] = mod
    antenv.axon_hooks = mod


def run(inputs: dict, trace: bool = False, **kw):
    if trace:
        _ensure_ntff_hook()
    if "nc" not in _CACHE:
        _CACHE["nc"] = _build_nc()
    nc = _CACHE["nc"]
    in_maps = _host_prep(inputs)
    res = run_bass_kernel_spmd(nc, in_maps, list(range(NCORES)), trace=trace, **kw)
    out = np.empty((B, B, H), np.float32)
    for c in range(NCORES):
        out[c * PB : (c + 1) * PB] = np.asarray(res.results[c]["out"], np.float32)
    return out, res


def kernel(**inputs) -> np.ndarray:
    out, _ = run(inputs)
    return out
